# revision 33
# baseline (speedup 1.0000x reference)
"""Causal self-attention (B=4, T=2048, C=1024, H=16, D=64) on 8 TRN2 NeuronCores.

Sharding: core = (batch b, head-group g) with b = core // 2, g = core % 2.
Each core computes heads [8g, 8g+8) of batch b and produces the partial
out-projection (C, T) fp16 for its head group; the host sums the two
head-group partials per batch and adds the output bias.

Speed structure (vs the all-fp16 baseline):
- Projections and scores run as fp8e4 DoubleRow matmuls (0.5 cycles/row,
  two k-tiles per pass). Weight accuracy is restored with a hi/lo split
  (hi = fp8(S*w), lo = fp8(S*w - hi)); weights are prescaled (SQ=32 for
  qkv, SV=8 for V) to clear fp8e4's 2^-9 subnormal floor. SQ is undone
  in the rope cos/sin tables; SV cancels in softmax normalization since
  the denominator ones-column carries the same scale.
- Scores use a "zero slot": lhsT k-group 1 points at a zeroed column
  block of the rq/rk tile, so a K=64 fp16-shaped matmul still gets the
  DoubleRow rate. q/k are cast to fp8 by the rope add.
- att@V: exp() writes fp8 straight from the Activation engine; V is
  stored as interleaved (v_hi, v_lo) fp8 pairs and one DoubleRow matmul
  computes v_hi^T e + v_lo^T e per s-chunk via a stride-0 broadcast of e.
  Diagonal s-chunks (which carry the concentrated softmax weight) use an
  fp16 e and an fp16 V copy instead - that one change cuts the output
  error ~2x while costing only the extra diagonal columns.
- Phases are emission-interleaved so the PE/DVE work of the V projection,
  next chunk's qk+rope, and the output projection hide under the
  Activation-engine-bound attention inner loop.
"""

import numpy as np

B, T, C = 4, 2048, 1024
H, D = 16, 64
N_CORES = 8
HPG = H // 2            # heads per core (group)
NCHUNK = 4              # head-pair chunks per core
KT = 8                  # k-tiles of 128 over C
TT = 4                  # t-tiles of 512 over T
NT = 512                # t tile (matmul N)
VS = 66                 # v column stride per head (64 dims + ones + pad)
VW = HPG * VS           # 528 v columns per k-chunk block
ROPE_BASE = 10000.0
SQ = 32.0               # qk weight prescale (undone via cs/css tables)
SV = 8.0                # v weight prescale (cancels in softmax normalization)

_CACHE = {}


def _build_nc():
    import concourse.bass as bass
    import concourse.tile as tile
    from concourse import bacc, mybir
    from contextlib import ExitStack

    f16 = mybir.dt.float16
    f32 = mybir.dt.float32
    f8 = mybir.dt.float8e4
    DR = mybir.MatmulPerfMode.DoubleRow

    nc = bacc.Bacc(
        "TRN2",
        target_bir_lowering=False,
        debug=False,
        enable_asserts=True,
        num_devices=N_CORES,
    )

    x8h_d = nc.dram_tensor("x8h", (10 * 128, T), f8, kind="ExternalInput").ap()
    x8l_d = nc.dram_tensor("x8l", (KT * 128, T), f8, kind="ExternalInput").ap()
    wqk8h_d = nc.dram_tensor("wqk8h", (128, 10 * 1024), f8, kind="ExternalInput").ap()
    wv8h_d = nc.dram_tensor("wv8h", (128, 10 * VW), f8, kind="ExternalInput").ap()
    wv8l_d = nc.dram_tensor("wv8l", (128, KT * VW), f8, kind="ExternalInput").ap()
    wo_d = nc.dram_tensor("wo", (128, NCHUNK * 1024), f16, kind="ExternalInput").ap()
    cs_d = nc.dram_tensor("cs", (128, T), f16, kind="ExternalInput").ap()
    css_d = nc.dram_tensor("css", (128, T), f16, kind="ExternalInput").ap()
    ot_d = nc.dram_tensor("ot", (1024, T), f16, kind="ExternalOutput").ap()

    SHUF = list(range(16, 32)) + list(range(0, 16))

    def slot_b(ap, w):
        # [K, 2, w] view with a stride-0 k-group dim (broadcast the same block)
        return bass.AP(ap.tensor, ap.offset, [list(ap.ap[0]), [0, 2], [1, w]])

    with tile.TileContext(nc) as tc:
        with ExitStack() as ctx, nc.allow_low_precision("fp8 attention pipeline"):
            consts = ctx.enter_context(tc.tile_pool(name="consts", bufs=1))
            qk_pool = ctx.enter_context(tc.tile_pool(name="qk", bufs=2))
            rtmp = ctx.enter_context(tc.tile_pool(name="rtmp", bufs=4))
            e8_pool = ctx.enter_context(tc.tile_pool(name="e8", bufs=6))
            e16_pool = ctx.enter_context(tc.tile_pool(name="e16", bufs=4))
            small = ctx.enter_context(tc.tile_pool(name="small", bufs=3))
            osb = ctx.enter_context(tc.tile_pool(name="osb", bufs=6))
            ps_big = ctx.enter_context(tc.tile_pool(name="psbig", bufs=2, space="PSUM"))
            ps_s = ctx.enter_context(tc.tile_pool(name="pss", bufs=2, space="PSUM"))
            ps_y = ctx.enter_context(tc.tile_pool(name="psy", bufs=1, space="PSUM"))

            # ---- resident tiles + input DMA ----
            # t-halved so the first vproj/rope chains start after half the load
            x8h = consts.tile([128, 10 * T], f8)
            x8l = consts.tile([128, KT * T], f8)
            for half in range(2):
                t0, t1 = half * (T // 2), (half + 1) * (T // 2)
                for kc in range(10):
                    nc.sync.dma_start(x8h[:, kc * T + t0: kc * T + t1],
                                      x8h_d[kc * 128:(kc + 1) * 128, t0:t1])
                for kc in range(KT):
                    nc.sync.dma_start(x8l[:, kc * T + t0: kc * T + t1],
                                      x8l_d[kc * 128:(kc + 1) * 128, t0:t1])

            def dma_split(dst, src, width, parts):
                step = width // parts
                for i in range(parts):
                    nc.sync.dma_start(dst[:, i * step:(i + 1) * step],
                                      src[:, i * step:(i + 1) * step])

            wv8h = consts.tile([128, 10 * VW], f8)
            dma_split(wv8h, wv8h_d, 10 * VW, 2)
            wv8l = consts.tile([128, KT * VW], f8)
            dma_split(wv8l, wv8l_d, KT * VW, 2)
            wqk8h = consts.tile([128, 10 * 1024], f8)
            dma_split(wqk8h, wqk8h_d, 10 * 1024, 5)
            cs = consts.tile([128, T], f16)
            dma_split(cs, cs_d, T, 2)
            css = consts.tile([128, T], f16)
            dma_split(css, css_d, T, 2)
            wo = consts.tile([128, NCHUNK * 1024], f16)
            dma_split(wo, wo_d, NCHUNK * 1024, 4)

            v8 = consts.tile([128, 16 * 2 * VW], f8)    # (m, hi|lo, col)
            v16 = consts.tile([128, 16 * VW], f16)      # fp16 V for diagonal chunks
            y_all = consts.tile([128, NCHUNK * T], f16)

            x8h3 = x8h[:].rearrange("p (kc t) -> p kc t", kc=10)
            x8l3 = x8l[:].rearrange("p (kc t) -> p kc t", kc=KT)
            wv8h3 = wv8h[:].rearrange("p (kc c) -> p kc c", kc=10)
            wv8l3 = wv8l[:].rearrange("p (kc c) -> p kc c", kc=KT)
            wqk8h3 = wqk8h[:].rearrange("p (kc c) -> p kc c", kc=10)
            v84 = v8[:].rearrange("p (m s c) -> p m s c", m=16, s=2)

            # ---- emission helpers (phases are interleaved for overlap) ----
            HVW = VW // 2   # 264: v columns for 4 heads incl. their ones cols

            def emit_vproj_m(m):
                # two half-width PSUM tiles split at the head-4 boundary so
                # vproj never touches the scores pool (ps_s)
                psa0 = ps_big.tile([128, HVW], f32, tag="big")
                psa1 = ps_big.tile([128, HVW], f32, tag="big")
                psa = (psa0, psa1)
                mm = []
                for j in range(KT // 2):
                    kk = slice(2 * j, 2 * j + 2)
                    mm.append((x8h3[:, kk, m * 128:(m + 1) * 128], wv8h3[:, kk, :]))
                    mm.append((x8l3[:, kk, m * 128:(m + 1) * 128], wv8h3[:, kk, :]))
                    mm.append((x8h3[:, kk, m * 128:(m + 1) * 128], wv8l3[:, kk, :]))
                mm.append((x8h3[:, 8:10, m * 128:(m + 1) * 128], wv8h3[:, 8:10, :]))
                n = len(mm)
                for i, (lhsT, rhs) in enumerate(mm):
                    for half in range(2):
                        nc.tensor.matmul(psa[half][:], lhsT,
                                         rhs[:, :, half * HVW:(half + 1) * HVW],
                                         start=(i == 0), stop=(i == n - 1),
                                         perf_mode=DR)
                base = m * 2 * VW
                for half in range(2):
                    o = half * HVW
                    nc.vector.tensor_copy(v8[:, base + o: base + o + HVW], psa[half][:])
                    nc.vector.tensor_copy(v16[:, m * VW + o: m * VW + o + HVW],
                                          psa[half][:])
                    nc.vector.tensor_tensor(
                        out=v8[:, base + VW + o: base + VW + o + HVW],
                        in0=psa[half][:], in1=v8[:, base + o: base + o + HVW],
                        op=mybir.AluOpType.subtract)

            qk_tiles = {}

            def ensure_qk(c):
                if c not in qk_tiles:
                    rq = qk_pool.tile([128, T + 128], f8, tag="rq")
                    rk = qk_pool.tile([128, T + 128], f8, tag="rk")
                    nc.gpsimd.memset(rq[:, T:T + 128], 0.0)
                    nc.gpsimd.memset(rk[:, T:T + 128], 0.0)
                    qk_tiles[c] = (rq, rk)
                return qk_tiles[c]

            def emit_rope_a(c, tt, which):
                """Projection + shuffle + the two rope muls (x2 on gpsimd)."""
                dst = ensure_qk(c)[which]
                t0 = tt * NT
                cm = c * 256 + which * 128
                ps = ps_big.tile([128, 512], f32, tag="big")
                mm = []
                for j in range(KT // 2):
                    kk = slice(2 * j, 2 * j + 2)
                    mm.append((wqk8h3[:, kk, cm:cm + 128], x8h3[:, kk, t0:t0 + NT]))
                    mm.append((wqk8h3[:, kk, cm:cm + 128], x8l3[:, kk, t0:t0 + NT]))
                mm.append((wqk8h3[:, 8:10, cm:cm + 128], x8h3[:, 8:10, t0:t0 + NT]))
                n = len(mm)
                for i, (lhsT, rhs) in enumerate(mm):
                    nc.tensor.matmul(ps[:], lhsT, rhs,
                                     start=(i == 0), stop=(i == n - 1), perf_mode=DR)
                s_t = rtmp.tile([128, 512], f32, tag="s")
                nc.vector.stream_shuffle(s_t[:], ps[:], SHUF)
                x1 = rtmp.tile([128, 512], f16, tag="x1")
                nc.vector.tensor_mul(x1[:], ps[:], cs[:, t0:t0 + NT])
                x2 = rtmp.tile([128, 512], f16, tag="x2")
                nc.gpsimd.tensor_mul(x2[:], s_t[:], css[:, t0:t0 + NT])
                return x1, x2

            def emit_rope_b(c, tt, which, x1, x2):
                """Deferred fp8 add (gpsimd) - emitted one instance behind
                stage a so the Pool queue never stalls the DVE queue."""
                dst = ensure_qk(c)[which]
                t0 = tt * NT
                nc.gpsimd.tensor_add(dst[:, t0:t0 + NT], x1[:], x2[:])

            def rope_items(c):
                """Pending-queue closures for all 8 rope instances of chunk c,
                with each add deferred two slots behind its producer."""
                items = []
                for tt in range(TT):
                    st = {}
                    for which in (0, 1):
                        def a_fn(c=c, tt=tt, which=which, st=st):
                            st[which] = emit_rope_a(c, tt, which)
                        items.append(a_fn)
                    for which in (0, 1):
                        def b_fn(c=c, tt=tt, which=which, st=st):
                            emit_rope_b(c, tt, which, *st[which])
                        items.append(b_fn)
                return items

            pending = []

            def emit_scores(c, tt, sc):
                rq, rk = qk_tiles[c]
                t0 = tt * NT
                s0 = sc * 128
                dlt = max(0, s0 - t0)
                w = NT - dlt
                sp = ps_s.tile([128, 1024], f32, tag="s")
                for h in range(2):
                    sl = rk[h * 64:(h + 1) * 64, s0:s0 + 128]
                    lhsT = bass.AP(sl.tensor, sl.offset,
                                   [list(sl.ap[0]), [T - s0, 2], [1, 128]])
                    sr = rq[h * 64:(h + 1) * 64, t0 + dlt:t0 + NT]
                    rhs = bass.AP(sr.tensor, sr.offset,
                                  [list(sr.ap[0]), [0, 2], [1, w]])
                    nc.tensor.matmul(sp[:, h * NT + dlt:(h + 1) * NT], lhsT, rhs,
                                     start=True, stop=True, perf_mode=DR,
                                     tile_position=(h * 64, 0))
                return sp

            def emit_exp(tt, sc, sp):
                t0 = tt * NT
                s0 = sc * 128
                dlt = max(0, s0 - t0)
                diag = s0 + 127 > t0
                e_t = (e16_pool if diag else e8_pool).tile(
                    [128, 1024], f16 if diag else f8)
                s3 = sp[:].rearrange("p (a b) -> p a b", a=2)[:, :, dlt:]
                e3 = e_t[:].rearrange("p (a b) -> p a b", a=2)[:, :, dlt:]
                nc.scalar.activation(e3, s3, mybir.ActivationFunctionType.Exp,
                                     bias=0.0, scale=0.125)
                if diag:
                    # keep iff j' >= p; for j' >= 128 that's always true,
                    # so only the first 128 columns need the select
                    e3m = e_t[:].rearrange("p (a b) -> p a b", a=2)[:, :, dlt:dlt + 128]
                    nc.gpsimd.affine_select(
                        out=e3m, in_=e3m,
                        compare_op=mybir.AluOpType.is_ge,
                        fill=0.0, base=0,
                        pattern=[[0, 2], [1, 128]], channel_multiplier=-1)
                return e_t

            def emit_attv(c, tt, sc, e_t, yp):
                t0 = tt * NT
                sc_max = (t0 + NT) // 128
                s0 = sc * 128
                dlt = max(0, s0 - t0)
                w = NT - dlt
                diag = s0 + 127 > t0
                for h in range(2):
                    vc = VS * (2 * c + h)
                    if diag:
                        nc.tensor.matmul(
                            yp[:, h * NT + dlt:(h + 1) * NT],
                            v16[:, sc * VW + vc: sc * VW + vc + 65],
                            e_t[:, h * NT + dlt:(h + 1) * NT],
                            start=(sc == 0), stop=(sc == sc_max - 1),
                            skip_group_check=True)
                    else:
                        nc.tensor.matmul(
                            yp[:, h * NT + dlt:(h + 1) * NT],
                            v84[:, sc, :, vc: vc + 65],
                            slot_b(e_t[:, h * NT + dlt: (h + 1) * NT], w),
                            start=(sc == 0), stop=(sc == sc_max - 1),
                            perf_mode=DR, skip_group_check=True)

            def emit_norm(c, tt, yp):
                t0 = tt * NT
                rd = small.tile([1, 1024], f16, tag="rd")
                nc.vector.reciprocal(rd[:], yp[64:65, :])
                rbc = small.tile([64, 1024], f16, tag="rbc")
                nc.gpsimd.partition_broadcast(rbc[:], rd[:])
                for h in range(2):
                    nc.vector.tensor_mul(
                        y_all[h * 64:(h + 1) * 64, c * T + t0: c * T + t0 + NT],
                        yp[0:64, h * NT:(h + 1) * NT],
                        rbc[:, h * NT:(h + 1) * NT])

            def emit_attn_chunk(c, on_norm=None):
                """Software-pipelined attention stream for one chunk: scores
                run one s-chunk ahead of att@V so the PE's in-order queue
                never starves the Activation engine's exp pipeline."""
                seq = [(tt, sc) for tt in range(TT)
                       for sc in range((tt + 1) * NT // 128)]
                sps = {}
                ets = {}
                yps = {}
                sps[0] = emit_scores(c, *seq[0])
                for i, (tt, sc) in enumerate(seq):
                    if sc == 0:
                        yp_t = ps_y.tile([65, 1024], f32, tag="y")
                        yps[tt] = yp_t
                    if i + 1 < len(seq):
                        sp_t = emit_scores(c, *seq[i + 1])
                        sps[i + 1] = sp_t
                    ets[i] = emit_exp(tt, sc, sps.pop(i))
                    emit_attv(c, tt, sc, ets.pop(i), yps[tt])
                    if pending:
                        pending.pop(0)()
                    if sc == (tt + 1) * NT // 128 - 1:
                        emit_norm(c, tt, yps.pop(tt))
                        if on_norm is not None:
                            on_norm(tt)

            def emit_oproj_ct(tt, ct):
                t0 = tt * NT
                po = ps_big.tile([128, 512], f32, tag="big")
                for c in range(NCHUNK):
                    nc.tensor.matmul(po[:], wo[:, c * 1024 + ct * 128: c * 1024 + ct * 128 + 128],
                                     y_all[:, c * T + t0: c * T + t0 + NT],
                                     start=(c == 0), stop=(c == NCHUNK - 1))
                ob = osb.tile([128, 512], f16)
                nc.vector.tensor_copy(ob[:], po[:])
                nc.sync.dma_start(ot_d[ct * 128:(ct + 1) * 128, t0:t0 + NT], ob[:])

            # ---- head phase: chunk-0 rope tt0 first, then first v-slices ----
            with nc.named_scope("head"):
                st = {}
                for which in (0, 1):
                    st[which] = emit_rope_a(0, 0, which)
                for which in (0, 1):
                    emit_rope_b(0, 0, which, *st[which])
                for m in range(4):
                    emit_vproj_m(m)
                st = {}
                for which in (0, 1):
                    st[which] = emit_rope_a(0, 1, which)
                for which in (0, 1):
                    emit_rope_b(0, 1, which, *st[which])

            # ---- pending helper work, drained one item per s-chunk of the
            # Act-bound attention inner loop (deadlines commented) ----
            rope0 = rope_items(0)[8:]   # chunk-0 tt2/tt3 (8 items)
            rope1 = rope_items(1)
            # c0 queue: vproj m4..m15 + rope(c0,tt2/tt3) + rope(c1).
            # m_k is popped >= (k-4) slots in; attV(c0) first reads m_k at
            # global slot >= k (4*tt slots precede the tt that reads it), and
            # rope(c0,tt2) B-items sit at index <= 11 < 12 slots before tt2.
            pending.extend([lambda m=m: emit_vproj_m(m) for m in (4, 5)])
            pending.extend(rope0[0:2])
            pending.extend([lambda m=m: emit_vproj_m(m) for m in (6, 7)])
            pending.extend(rope0[2:4])
            pending.extend([lambda m=m: emit_vproj_m(m) for m in (8, 9)])
            pending.extend(rope0[4:6])
            pending.extend([lambda m=m: emit_vproj_m(m) for m in (10, 11)])
            pending.extend(rope0[6:8])
            pending.extend([lambda m=m: emit_vproj_m(m) for m in (12, 13, 14, 15)])
            pending.extend(rope1)

            def queue_oproj(tt):
                if tt < 3:  # tt3 runs in the tail
                    pending.extend(
                        [lambda ct=ct, tt=tt: emit_oproj_ct(tt, ct)
                         for ct in range(8)])

            for c in range(NCHUNK):
                with nc.named_scope(f"attn{c}"):
                    if c in (1, 2):
                        pending.extend(rope_items(c + 1))
                    emit_attn_chunk(
                        c, on_norm=queue_oproj if c == NCHUNK - 1 else None)

            with nc.named_scope("tail"):
                while pending:
                    pending.pop(0)()
                for ct in range(8):
                    emit_oproj_ct(3, ct)

    nc.compile()
    return nc


def _fp8_split(a, np8, scale=1.0):
    """scale*a -> (hi, lo) fp8 with hi + lo ~= scale*a to ~0.13%.

    The scale lifts 0.02-magnitude weights above fp8e4's 2^-9 subnormal
    floor so the lo residual can actually represent the hi rounding error.
    """
    a = np.asarray(a, dtype=np.float32) * scale
    hi = a.astype(np8)
    lo = (a - hi.astype(np.float32)).astype(np8)
    return hi, lo


def _prep_inputs(x, qkv_w, qkv_b):
    """Build the per-core input maps (all host-side numpy)."""
    from concourse import mybir
    np8 = mybir.dt.np(mybir.dt.float8e4)

    x = np.asarray(x, dtype=np.float32)
    qkv_w = np.asarray(qkv_w, dtype=np.float32)
    qkv_b = np.asarray(qkv_b, dtype=np.float32)

    # x8 per batch: hi [10*128, T] (ktiles 8/9 = ones row), lo [8*128, T]
    x8hs, x8ls = [], []
    for b in range(B):
        xh = np.zeros((10 * 128, T), dtype=np8)
        xl = np.zeros((KT * 128, T), dtype=np8)
        hi, lo = _fp8_split(x[b].T, np8)
        xh[:C] = hi
        xl[:C] = lo
        xh[C] = np8(1.0)        # aug ktile 8: ones row
        xh[9 * 128] = np8(1.0)  # aug ktile 9: duplicate ones row
        x8hs.append(xh)
        x8ls.append(xl)

    r = np.arange(64)
    d_r = 2 * ((r // 32) * 16 + (r % 16)) + ((r % 32) >= 16)  # row -> head dim
    p = np.arange(128)
    f_p = ((p // 32) % 2) * 16 + (p % 16)

    ins_g = []
    for g in range(2):
        # wqk8h: [p, kc*1024 + c*256 + which*128 + m]; kc8/9 = bias hi/lo on row 0
        wqkh = np.zeros((128, 10 * 1024), dtype=np8)
        for c in range(NCHUNK):
            for which in range(2):  # 0=q, 1=k
                rows = np.concatenate([
                    which * C + (8 * g + 2 * c + hh) * 64 + d_r for hh in range(2)
                ])  # 128 feature rows
                blk = qkv_w[rows, :]          # (128 feat, 1024 k)
                cm = c * 256 + which * 128
                for kc in range(KT):
                    hi, _lo = _fp8_split(blk[:, kc * 128:(kc + 1) * 128].T, np8, SQ)
                    wqkh[:, kc * 1024 + cm: kc * 1024 + cm + 128] = hi
                bh, bl = _fp8_split(qkv_b[rows], np8, SQ)
                wqkh[0, 8 * 1024 + cm: 8 * 1024 + cm + 128] = bh
                wqkh[0, 9 * 1024 + cm: 9 * 1024 + cm + 128] = bl
        # wv8: [p, kc*VW + VS*h + j]; kc8 = aug hi (bias+ones), kc9 = aug lo (bias)
        wva = np.zeros((KT * 128, VW), dtype=np.float32)
        aug = np.zeros((128, VW), dtype=np.float32)
        for h in range(HPG):
            rows = 2 * C + (8 * g + h) * 64 + np.arange(64)
            wva[:, VS * h: VS * h + 64] = qkv_w[rows, :].T
            aug[0, VS * h: VS * h + 64] = qkv_b[rows]
            aug[0, VS * h + 64] = 1.0
        wvh = np.zeros((128, 10 * VW), dtype=np8)
        wvl = np.zeros((128, KT * VW), dtype=np8)
        for kc in range(KT):
            hi, lo = _fp8_split(wva[kc * 128:(kc + 1) * 128], np8, SV)
            wvh[:, kc * VW:(kc + 1) * VW] = hi
            wvl[:, kc * VW:(kc + 1) * VW] = lo
        augh, augl = _fp8_split(aug, np8, SV)  # ones col becomes SV (exact in fp8)
        augl[0, VS * np.arange(HPG) + 64] = np8(0.0)  # ones col only in hi
        wvh[:, 8 * VW: 9 * VW] = augh
        wvh[:, 9 * VW: 10 * VW] = augl
        ins_g.append((wqkh, wvh, wvl))

    # rope tables (divided by SQ to undo the qk weight prescale)
    inv_freq = (1.0 / (ROPE_BASE ** (np.arange(0, D, 2) / D))).astype(np.float64)
    t = np.arange(T, dtype=np.float64)
    ang = t[None, :] * inv_freq[f_p][:, None]          # (128, T)
    cs = (np.cos(ang) / SQ).astype(np.float16)
    sgn = np.where((p % 32) < 16, -1.0, 1.0)[:, None]
    css = (sgn * np.sin(ang) / SQ).astype(np.float16)

    return x8hs, x8ls, ins_g, cs, css


def _prep_wo(out_w, g):
    out_w = np.asarray(out_w, dtype=np.float32)
    wo = np.empty((128, NCHUNK * 1024), dtype=np.float16)
    for c in range(NCHUNK):
        rows = np.concatenate([(8 * g + 2 * c + hh) * 64 + np.arange(64) for hh in range(2)])
        wo[:, c * 1024:(c + 1) * 1024] = out_w[:, rows].astype(np.float16).T
    return wo


def _build_in_maps(x, qkv_w, qkv_b, out_w):
    x8hs, x8ls, ins_g, cs, css = _prep_inputs(x, qkv_w, qkv_b)
    wos = [_prep_wo(out_w, g) for g in range(2)]
    in_maps = []
    for core in range(N_CORES):
        b, g = core // 2, core % 2
        wqkh, wvh, wvl = ins_g[g]
        in_maps.append({
            "x8h": x8hs[b], "x8l": x8ls[b],
            "wqk8h": wqkh,
            "wv8h": wvh, "wv8l": wvl,
            "wo": wos[g], "cs": cs, "css": css,
        })
    return in_maps


def kernel(x, qkv_w, qkv_b, out_w, out_b):
    from concourse.bass_utils import run_bass_kernel_spmd

    if "nc" not in _CACHE:
        _CACHE["nc"] = _build_nc()
    nc = _CACHE["nc"]

    in_maps = _build_in_maps(x, qkv_w, qkv_b, out_w)
    out_b = np.asarray(out_b, dtype=np.float32)

    try:
        res = run_bass_kernel_spmd(nc, in_maps, core_ids=list(range(N_CORES)))
    except ModuleNotFoundError:
        # BASS_TRACE set but the NTFF profile hook isn't importable here
        import os
        os.environ["BASS_NEVER_TRACE"] = "1"
        res = run_bass_kernel_spmd(nc, in_maps, core_ids=list(range(N_CORES)))

    out = np.empty((B, T, C), dtype=np.float32)
    for b in range(B):
        pt = (res.results[2 * b]["ot"].astype(np.float32)
              + res.results[2 * b + 1]["ot"].astype(np.float32))  # (C, T)
        out[b] = pt.T + out_b[None, :]
    return out


# revision 35
# speedup vs baseline: 1.0606x; 1.0606x over previous
"""Causal self-attention (B=4, T=2048, C=1024, H=16, D=64) on 8 TRN2 NeuronCores.

Sharding: core = (batch b, head-group g) with b = core // 2, g = core % 2.
Each core computes heads [8g, 8g+8) of batch b and produces the partial
out-projection (C, T) fp16 for its head group; the host sums the two
head-group partials per batch and adds the output bias.

Speed structure (vs the all-fp16 baseline):
- Projections and scores run as fp8e4 DoubleRow matmuls (0.5 cycles/row,
  two k-tiles per pass). Weight accuracy is restored with a hi/lo split
  (hi = fp8(S*w), lo = fp8(S*w - hi)); weights are prescaled (SQ=32 for
  qkv, SV=8 for V) to clear fp8e4's 2^-9 subnormal floor. SQ is undone
  in the rope cos/sin tables; SV cancels in softmax normalization since
  the denominator ones-column carries the same scale.
- Scores use a "zero slot": lhsT k-group 1 points at a zeroed column
  block of the rq/rk tile, so a K=64 fp16-shaped matmul still gets the
  DoubleRow rate. q/k are cast to fp8 by the rope add.
- att@V: exp() writes fp8 straight from the Activation engine; V is
  stored as interleaved (v_hi, v_lo) fp8 pairs and one DoubleRow matmul
  computes v_hi^T e + v_lo^T e per s-chunk via a stride-0 broadcast of e.
  Diagonal s-chunks (which carry the concentrated softmax weight) use an
  fp16 e and an fp16 V copy instead - that one change cuts the output
  error ~2x while costing only the extra diagonal columns.
- Phases are emission-interleaved so the PE/DVE work of the V projection,
  next chunk's qk+rope, and the output projection hide under the
  Activation-engine-bound attention inner loop.
"""

import numpy as np

B, T, C = 4, 2048, 1024
H, D = 16, 64
N_CORES = 8
HPG = H // 2            # heads per core (group)
NCHUNK = 4              # head-pair chunks per core
KT = 8                  # k-tiles of 128 over C
TT = 4                  # t-tiles of 512 over T
NT = 512                # t tile (matmul N)
VS = 66                 # v column stride per head (64 dims + ones + pad)
VW = HPG * VS           # 528 v columns per k-chunk block
ROPE_BASE = 10000.0
SQ = 32.0               # qk weight prescale (undone via cs/css tables)
SV = 8.0                # v weight prescale (cancels in softmax normalization)

_CACHE = {}


def _build_nc():
    import concourse.bass as bass
    import concourse.tile as tile
    from concourse import bacc, mybir
    from contextlib import ExitStack

    f16 = mybir.dt.float16
    f32 = mybir.dt.float32
    f8 = mybir.dt.float8e4
    DR = mybir.MatmulPerfMode.DoubleRow

    nc = bacc.Bacc(
        "TRN2",
        target_bir_lowering=False,
        debug=False,
        enable_asserts=True,
        num_devices=N_CORES,
    )

    x8h_d = nc.dram_tensor("x8h", (10 * 128, T), f8, kind="ExternalInput").ap()
    x8l_d = nc.dram_tensor("x8l", (KT * 128, T), f8, kind="ExternalInput").ap()
    wqk8h_d = nc.dram_tensor("wqk8h", (128, 10 * 1024), f8, kind="ExternalInput").ap()
    wv8h_d = nc.dram_tensor("wv8h", (128, 10 * VW), f8, kind="ExternalInput").ap()
    wv8l_d = nc.dram_tensor("wv8l", (128, KT * VW), f8, kind="ExternalInput").ap()
    wo_d = nc.dram_tensor("wo", (128, NCHUNK * 1024), f16, kind="ExternalInput").ap()
    cs_d = nc.dram_tensor("cs", (128, T), f16, kind="ExternalInput").ap()
    css_d = nc.dram_tensor("css", (128, T), f16, kind="ExternalInput").ap()
    ot_d = nc.dram_tensor("ot", (1024, T), f16, kind="ExternalOutput").ap()

    SHUF = list(range(16, 32)) + list(range(0, 16))

    def slot_b(ap, w):
        # [K, 2, w] view with a stride-0 k-group dim (broadcast the same block)
        return bass.AP(ap.tensor, ap.offset, [list(ap.ap[0]), [0, 2], [1, w]])

    with tile.TileContext(nc) as tc:
        with ExitStack() as ctx, nc.allow_low_precision("fp8 attention pipeline"):
            consts = ctx.enter_context(tc.tile_pool(name="consts", bufs=1))
            qk_pool = ctx.enter_context(tc.tile_pool(name="qk", bufs=2))
            rtmp = ctx.enter_context(tc.tile_pool(name="rtmp", bufs=4))
            e8_pool = ctx.enter_context(tc.tile_pool(name="e8", bufs=6))
            e16_pool = ctx.enter_context(tc.tile_pool(name="e16", bufs=4))
            small = ctx.enter_context(tc.tile_pool(name="small", bufs=3))
            osb = ctx.enter_context(tc.tile_pool(name="osb", bufs=6))
            ps_big = ctx.enter_context(tc.tile_pool(name="psbig", bufs=2, space="PSUM"))
            ps_s = ctx.enter_context(tc.tile_pool(name="pss", bufs=2, space="PSUM"))
            ps_y = ctx.enter_context(tc.tile_pool(name="psy", bufs=1, space="PSUM"))

            # ---- resident tiles + input DMA ----
            # issue order follows first use: qk weights + x (t-half 0) feed the
            # head rope chain, then wv for vproj, then the rest
            x8h = consts.tile([128, 10 * T], f8)
            x8l = consts.tile([128, KT * T], f8)
            wqk8h = consts.tile([128, 10 * 1024], f8)
            wv8h = consts.tile([128, 10 * VW], f8)
            wv8l = consts.tile([128, KT * VW], f8)
            cs = consts.tile([128, T], f16)
            css = consts.tile([128, T], f16)
            wo = consts.tile([128, NCHUNK * 1024], f16)

            def dma_split(dst, src, width, parts):
                step = width // parts
                for i in range(parts):
                    nc.sync.dma_start(dst[:, i * step:(i + 1) * step],
                                      src[:, i * step:(i + 1) * step])

            def x_half(half):
                t0, t1 = half * (T // 2), (half + 1) * (T // 2)
                for kc in range(10):
                    nc.sync.dma_start(x8h[:, kc * T + t0: kc * T + t1],
                                      x8h_d[kc * 128:(kc + 1) * 128, t0:t1])
                for kc in range(KT):
                    nc.sync.dma_start(x8l[:, kc * T + t0: kc * T + t1],
                                      x8l_d[kc * 128:(kc + 1) * 128, t0:t1])

            dma_split(wqk8h, wqk8h_d, 10 * 1024, 5)
            x_half(0)
            dma_split(cs, cs_d, T, 2)
            dma_split(css, css_d, T, 2)
            dma_split(wv8h, wv8h_d, 10 * VW, 2)
            dma_split(wv8l, wv8l_d, KT * VW, 2)
            x_half(1)
            dma_split(wo, wo_d, NCHUNK * 1024, 4)

            v8 = consts.tile([128, 16 * 2 * VW], f8)    # (m, hi|lo, col)
            v16 = consts.tile([128, 16 * VW], f16)      # fp16 V for diagonal chunks
            y_all = consts.tile([128, NCHUNK * T], f16)

            x8h3 = x8h[:].rearrange("p (kc t) -> p kc t", kc=10)
            x8l3 = x8l[:].rearrange("p (kc t) -> p kc t", kc=KT)
            wv8h3 = wv8h[:].rearrange("p (kc c) -> p kc c", kc=10)
            wv8l3 = wv8l[:].rearrange("p (kc c) -> p kc c", kc=KT)
            wqk8h3 = wqk8h[:].rearrange("p (kc c) -> p kc c", kc=10)
            v84 = v8[:].rearrange("p (m s c) -> p m s c", m=16, s=2)

            # ---- emission helpers (phases are interleaved for overlap) ----
            HVW = VW // 2   # 264: v columns for 4 heads incl. their ones cols

            def emit_vproj_m(m):
                # two half-width PSUM tiles split at the head-4 boundary so
                # vproj never touches the scores pool (ps_s)
                psa0 = ps_big.tile([128, HVW], f32, tag="big")
                psa1 = ps_big.tile([128, HVW], f32, tag="big")
                psa = (psa0, psa1)
                mm = []
                for j in range(KT // 2):
                    kk = slice(2 * j, 2 * j + 2)
                    mm.append((x8h3[:, kk, m * 128:(m + 1) * 128], wv8h3[:, kk, :]))
                    mm.append((x8l3[:, kk, m * 128:(m + 1) * 128], wv8h3[:, kk, :]))
                    mm.append((x8h3[:, kk, m * 128:(m + 1) * 128], wv8l3[:, kk, :]))
                mm.append((x8h3[:, 8:10, m * 128:(m + 1) * 128], wv8h3[:, 8:10, :]))
                n = len(mm)
                for i, (lhsT, rhs) in enumerate(mm):
                    for half in range(2):
                        nc.tensor.matmul(psa[half][:], lhsT,
                                         rhs[:, :, half * HVW:(half + 1) * HVW],
                                         start=(i == 0), stop=(i == n - 1),
                                         perf_mode=DR)
                base = m * 2 * VW
                for half in range(2):
                    o = half * HVW
                    nc.vector.tensor_copy(v8[:, base + o: base + o + HVW], psa[half][:])
                    nc.vector.tensor_copy(v16[:, m * VW + o: m * VW + o + HVW],
                                          psa[half][:])
                    nc.vector.tensor_tensor(
                        out=v8[:, base + VW + o: base + VW + o + HVW],
                        in0=psa[half][:], in1=v8[:, base + o: base + o + HVW],
                        op=mybir.AluOpType.subtract)

            qk_tiles = {}

            def ensure_qk(c):
                if c not in qk_tiles:
                    rq = qk_pool.tile([128, T + 128], f8, tag="rq")
                    rk = qk_pool.tile([128, T + 128], f8, tag="rk")
                    nc.gpsimd.memset(rq[:, T:T + 128], 0.0)
                    nc.gpsimd.memset(rk[:, T:T + 128], 0.0)
                    qk_tiles[c] = (rq, rk)
                return qk_tiles[c]

            def emit_rope_a(c, tt, which):
                """Projection + shuffle + the two rope muls (x2 on gpsimd)."""
                dst = ensure_qk(c)[which]
                t0 = tt * NT
                cm = c * 256 + which * 128
                ps = ps_big.tile([128, 512], f32, tag="big")
                mm = []
                for j in range(KT // 2):
                    kk = slice(2 * j, 2 * j + 2)
                    mm.append((wqk8h3[:, kk, cm:cm + 128], x8h3[:, kk, t0:t0 + NT]))
                    mm.append((wqk8h3[:, kk, cm:cm + 128], x8l3[:, kk, t0:t0 + NT]))
                mm.append((wqk8h3[:, 8:10, cm:cm + 128], x8h3[:, 8:10, t0:t0 + NT]))
                n = len(mm)
                for i, (lhsT, rhs) in enumerate(mm):
                    nc.tensor.matmul(ps[:], lhsT, rhs,
                                     start=(i == 0), stop=(i == n - 1), perf_mode=DR)
                s_t = rtmp.tile([128, 512], f32, tag="s")
                nc.vector.stream_shuffle(s_t[:], ps[:], SHUF)
                x1 = rtmp.tile([128, 512], f16, tag="x1")
                nc.vector.tensor_mul(x1[:], ps[:], cs[:, t0:t0 + NT])
                x2 = rtmp.tile([128, 512], f16, tag="x2")
                nc.gpsimd.tensor_mul(x2[:], s_t[:], css[:, t0:t0 + NT])
                return x1, x2

            def emit_rope_b(c, tt, which, x1, x2):
                """Deferred fp8 add (gpsimd) - emitted one instance behind
                stage a so the Pool queue never stalls the DVE queue."""
                dst = ensure_qk(c)[which]
                t0 = tt * NT
                nc.gpsimd.tensor_add(dst[:, t0:t0 + NT], x1[:], x2[:])

            def rope_items(c):
                """Pending-queue closures for all 8 rope instances of chunk c,
                with each add deferred two slots behind its producer."""
                items = []
                for tt in range(TT):
                    st = {}
                    for which in (0, 1):
                        def a_fn(c=c, tt=tt, which=which, st=st):
                            st[which] = emit_rope_a(c, tt, which)
                        items.append(a_fn)
                    for which in (0, 1):
                        def b_fn(c=c, tt=tt, which=which, st=st):
                            emit_rope_b(c, tt, which, *st[which])
                        items.append(b_fn)
                return items

            pending = []

            def emit_scores(c, tt, sc):
                rq, rk = qk_tiles[c]
                t0 = tt * NT
                s0 = sc * 128
                dlt = max(0, s0 - t0)
                w = NT - dlt
                sp = ps_s.tile([128, 1024], f32, tag="s")
                for h in range(2):
                    sl = rk[h * 64:(h + 1) * 64, s0:s0 + 128]
                    lhsT = bass.AP(sl.tensor, sl.offset,
                                   [list(sl.ap[0]), [T - s0, 2], [1, 128]])
                    sr = rq[h * 64:(h + 1) * 64, t0 + dlt:t0 + NT]
                    rhs = bass.AP(sr.tensor, sr.offset,
                                  [list(sr.ap[0]), [0, 2], [1, w]])
                    nc.tensor.matmul(sp[:, h * NT + dlt:(h + 1) * NT], lhsT, rhs,
                                     start=True, stop=True, perf_mode=DR,
                                     tile_position=(h * 64, 0))
                return sp

            def emit_exp(tt, sc, sp):
                t0 = tt * NT
                s0 = sc * 128
                dlt = max(0, s0 - t0)
                diag = s0 + 127 > t0
                e_t = (e16_pool if diag else e8_pool).tile(
                    [128, 1024], f16 if diag else f8)
                s3 = sp[:].rearrange("p (a b) -> p a b", a=2)[:, :, dlt:]
                e3 = e_t[:].rearrange("p (a b) -> p a b", a=2)[:, :, dlt:]
                nc.scalar.activation(e3, s3, mybir.ActivationFunctionType.Exp,
                                     bias=0.0, scale=0.125)
                if diag:
                    # keep iff j' >= p; for j' >= 128 that's always true,
                    # so only the first 128 columns need the select
                    e3m = e_t[:].rearrange("p (a b) -> p a b", a=2)[:, :, dlt:dlt + 128]
                    nc.gpsimd.affine_select(
                        out=e3m, in_=e3m,
                        compare_op=mybir.AluOpType.is_ge,
                        fill=0.0, base=0,
                        pattern=[[0, 2], [1, 128]], channel_multiplier=-1)
                return e_t

            def emit_attv(c, tt, sc, e_t, yp):
                t0 = tt * NT
                sc_max = (t0 + NT) // 128
                s0 = sc * 128
                dlt = max(0, s0 - t0)
                w = NT - dlt
                diag = s0 + 127 > t0
                for h in range(2):
                    vc = VS * (2 * c + h)
                    if diag:
                        nc.tensor.matmul(
                            yp[:, h * NT + dlt:(h + 1) * NT],
                            v16[:, sc * VW + vc: sc * VW + vc + 65],
                            e_t[:, h * NT + dlt:(h + 1) * NT],
                            start=(sc == 0), stop=(sc == sc_max - 1),
                            skip_group_check=True)
                    else:
                        nc.tensor.matmul(
                            yp[:, h * NT + dlt:(h + 1) * NT],
                            v84[:, sc, :, vc: vc + 65],
                            slot_b(e_t[:, h * NT + dlt: (h + 1) * NT], w),
                            start=(sc == 0), stop=(sc == sc_max - 1),
                            perf_mode=DR, skip_group_check=True)

            def emit_norm(c, tt, yp):
                t0 = tt * NT
                rd = small.tile([1, 1024], f16, tag="rd")
                nc.vector.reciprocal(rd[:], yp[64:65, :])
                rbc = small.tile([64, 1024], f16, tag="rbc")
                nc.gpsimd.partition_broadcast(rbc[:], rd[:])
                for h in range(2):
                    nc.vector.tensor_mul(
                        y_all[h * 64:(h + 1) * 64, c * T + t0: c * T + t0 + NT],
                        yp[0:64, h * NT:(h + 1) * NT],
                        rbc[:, h * NT:(h + 1) * NT])

            def emit_attn_chunk(c, on_norm=None):
                """Software-pipelined attention stream for one chunk: scores
                run one s-chunk ahead of att@V so the PE's in-order queue
                never starves the Activation engine's exp pipeline."""
                seq = [(tt, sc) for tt in range(TT)
                       for sc in range((tt + 1) * NT // 128)]
                sps = {}
                ets = {}
                yps = {}
                sps[0] = emit_scores(c, *seq[0])

                def attv_and_norm(j):
                    tt_j, sc_j = seq[j]
                    if sc_j == 0:
                        yp_t = ps_y.tile([65, 1024], f32, tag="y")
                        yps[tt_j] = yp_t
                    emit_attv(c, tt_j, sc_j, ets.pop(j), yps[tt_j])
                    if sc_j == (tt_j + 1) * NT // 128 - 1:
                        emit_norm(c, tt_j, yps.pop(tt_j))
                        if on_norm is not None:
                            on_norm(tt_j)

                for i, (tt, sc) in enumerate(seq):
                    if i + 1 < len(seq):
                        sp_t = emit_scores(c, *seq[i + 1])
                        sps[i + 1] = sp_t
                    ets[i] = emit_exp(tt, sc, sps.pop(i))
                    if i >= 1:
                        attv_and_norm(i - 1)  # att@V one s-chunk behind exp
                    if pending:
                        pending.pop(0)()
                for i in (len(seq) - 1,):
                    attv_and_norm(i)

            def emit_oproj_ct(tt, ct):
                t0 = tt * NT
                po = ps_big.tile([128, 512], f32, tag="big")
                for c in range(NCHUNK):
                    nc.tensor.matmul(po[:], wo[:, c * 1024 + ct * 128: c * 1024 + ct * 128 + 128],
                                     y_all[:, c * T + t0: c * T + t0 + NT],
                                     start=(c == 0), stop=(c == NCHUNK - 1))
                ob = osb.tile([128, 512], f16)
                nc.vector.tensor_copy(ob[:], po[:])
                nc.sync.dma_start(ot_d[ct * 128:(ct + 1) * 128, t0:t0 + NT], ob[:])

            # ---- head phase: chunk-0 rope tt0 first, then first v-slices ----
            with nc.named_scope("head"):
                st = {}
                for which in (0, 1):
                    st[which] = emit_rope_a(0, 0, which)
                for which in (0, 1):
                    emit_rope_b(0, 0, which, *st[which])
                for m in range(4):
                    emit_vproj_m(m)
                st = {}
                for which in (0, 1):
                    st[which] = emit_rope_a(0, 1, which)
                for which in (0, 1):
                    emit_rope_b(0, 1, which, *st[which])

            # ---- pending helper work, drained one item per s-chunk of the
            # Act-bound attention inner loop (deadlines commented) ----
            rope0 = rope_items(0)[8:]   # chunk-0 tt2/tt3 (8 items)
            rope1 = rope_items(1)
            # c0 queue: vproj m4..m15 + rope(c0,tt2/tt3) + rope(c1).
            # m_k is popped >= (k-4) slots in; attV(c0) first reads m_k at
            # global slot >= k (4*tt slots precede the tt that reads it), and
            # rope(c0,tt2) B-items sit at index <= 11 < 12 slots before tt2.
            pending.extend([lambda m=m: emit_vproj_m(m) for m in (4, 5)])
            pending.extend(rope0[0:2])
            pending.extend([lambda m=m: emit_vproj_m(m) for m in (6, 7)])
            pending.extend(rope0[2:4])
            pending.extend([lambda m=m: emit_vproj_m(m) for m in (8, 9)])
            pending.extend(rope0[4:6])
            pending.extend([lambda m=m: emit_vproj_m(m) for m in (10, 11)])
            pending.extend(rope0[6:8])
            pending.extend([lambda m=m: emit_vproj_m(m) for m in (12, 13, 14, 15)])
            pending.extend(rope1)

            def queue_oproj(tt):
                if tt < 3:  # tt3 runs in the tail
                    pending.extend(
                        [lambda ct=ct, tt=tt: emit_oproj_ct(tt, ct)
                         for ct in range(8)])

            for c in range(NCHUNK):
                with nc.named_scope(f"attn{c}"):
                    if c in (1, 2):
                        pending.extend(rope_items(c + 1))
                    emit_attn_chunk(
                        c, on_norm=queue_oproj if c == NCHUNK - 1 else None)

            with nc.named_scope("tail"):
                while pending:
                    pending.pop(0)()
                for ct in range(8):
                    emit_oproj_ct(3, ct)

    nc.compile()
    return nc


def _fp8_split(a, np8, scale=1.0):
    """scale*a -> (hi, lo) fp8 with hi + lo ~= scale*a to ~0.13%.

    The scale lifts 0.02-magnitude weights above fp8e4's 2^-9 subnormal
    floor so the lo residual can actually represent the hi rounding error.
    """
    a = np.asarray(a, dtype=np.float32) * scale
    hi = a.astype(np8)
    lo = (a - hi.astype(np.float32)).astype(np8)
    return hi, lo


def _prep_inputs(x, qkv_w, qkv_b):
    """Build the per-core input maps (all host-side numpy)."""
    from concourse import mybir
    np8 = mybir.dt.np(mybir.dt.float8e4)

    x = np.asarray(x, dtype=np.float32)
    qkv_w = np.asarray(qkv_w, dtype=np.float32)
    qkv_b = np.asarray(qkv_b, dtype=np.float32)

    # x8 per batch: hi [10*128, T] (ktiles 8/9 = ones row), lo [8*128, T]
    x8hs, x8ls = [], []
    for b in range(B):
        xh = np.zeros((10 * 128, T), dtype=np8)
        xl = np.zeros((KT * 128, T), dtype=np8)
        hi, lo = _fp8_split(x[b].T, np8)
        xh[:C] = hi
        xl[:C] = lo
        xh[C] = np8(1.0)        # aug ktile 8: ones row
        xh[9 * 128] = np8(1.0)  # aug ktile 9: duplicate ones row
        x8hs.append(xh)
        x8ls.append(xl)

    r = np.arange(64)
    d_r = 2 * ((r // 32) * 16 + (r % 16)) + ((r % 32) >= 16)  # row -> head dim
    p = np.arange(128)
    f_p = ((p // 32) % 2) * 16 + (p % 16)

    ins_g = []
    for g in range(2):
        # wqk8h: [p, kc*1024 + c*256 + which*128 + m]; kc8/9 = bias hi/lo on row 0
        wqkh = np.zeros((128, 10 * 1024), dtype=np8)
        for c in range(NCHUNK):
            for which in range(2):  # 0=q, 1=k
                rows = np.concatenate([
                    which * C + (8 * g + 2 * c + hh) * 64 + d_r for hh in range(2)
                ])  # 128 feature rows
                blk = qkv_w[rows, :]          # (128 feat, 1024 k)
                cm = c * 256 + which * 128
                for kc in range(KT):
                    hi, _lo = _fp8_split(blk[:, kc * 128:(kc + 1) * 128].T, np8, SQ)
                    wqkh[:, kc * 1024 + cm: kc * 1024 + cm + 128] = hi
                bh, bl = _fp8_split(qkv_b[rows], np8, SQ)
                wqkh[0, 8 * 1024 + cm: 8 * 1024 + cm + 128] = bh
                wqkh[0, 9 * 1024 + cm: 9 * 1024 + cm + 128] = bl
        # wv8: [p, kc*VW + VS*h + j]; kc8 = aug hi (bias+ones), kc9 = aug lo (bias)
        wva = np.zeros((KT * 128, VW), dtype=np.float32)
        aug = np.zeros((128, VW), dtype=np.float32)
        for h in range(HPG):
            rows = 2 * C + (8 * g + h) * 64 + np.arange(64)
            wva[:, VS * h: VS * h + 64] = qkv_w[rows, :].T
            aug[0, VS * h: VS * h + 64] = qkv_b[rows]
            aug[0, VS * h + 64] = 1.0
        wvh = np.zeros((128, 10 * VW), dtype=np8)
        wvl = np.zeros((128, KT * VW), dtype=np8)
        for kc in range(KT):
            hi, lo = _fp8_split(wva[kc * 128:(kc + 1) * 128], np8, SV)
            wvh[:, kc * VW:(kc + 1) * VW] = hi
            wvl[:, kc * VW:(kc + 1) * VW] = lo
        augh, augl = _fp8_split(aug, np8, SV)  # ones col becomes SV (exact in fp8)
        augl[0, VS * np.arange(HPG) + 64] = np8(0.0)  # ones col only in hi
        wvh[:, 8 * VW: 9 * VW] = augh
        wvh[:, 9 * VW: 10 * VW] = augl
        ins_g.append((wqkh, wvh, wvl))

    # rope tables (divided by SQ to undo the qk weight prescale)
    inv_freq = (1.0 / (ROPE_BASE ** (np.arange(0, D, 2) / D))).astype(np.float64)
    t = np.arange(T, dtype=np.float64)
    ang = t[None, :] * inv_freq[f_p][:, None]          # (128, T)
    cs = (np.cos(ang) / SQ).astype(np.float16)
    sgn = np.where((p % 32) < 16, -1.0, 1.0)[:, None]
    css = (sgn * np.sin(ang) / SQ).astype(np.float16)

    return x8hs, x8ls, ins_g, cs, css


def _prep_wo(out_w, g):
    out_w = np.asarray(out_w, dtype=np.float32)
    wo = np.empty((128, NCHUNK * 1024), dtype=np.float16)
    for c in range(NCHUNK):
        rows = np.concatenate([(8 * g + 2 * c + hh) * 64 + np.arange(64) for hh in range(2)])
        wo[:, c * 1024:(c + 1) * 1024] = out_w[:, rows].astype(np.float16).T
    return wo


def _build_in_maps(x, qkv_w, qkv_b, out_w):
    x8hs, x8ls, ins_g, cs, css = _prep_inputs(x, qkv_w, qkv_b)
    wos = [_prep_wo(out_w, g) for g in range(2)]
    in_maps = []
    for core in range(N_CORES):
        b, g = core // 2, core % 2
        wqkh, wvh, wvl = ins_g[g]
        in_maps.append({
            "x8h": x8hs[b], "x8l": x8ls[b],
            "wqk8h": wqkh,
            "wv8h": wvh, "wv8l": wvl,
            "wo": wos[g], "cs": cs, "css": css,
        })
    return in_maps


def kernel(x, qkv_w, qkv_b, out_w, out_b):
    from concourse.bass_utils import run_bass_kernel_spmd

    if "nc" not in _CACHE:
        _CACHE["nc"] = _build_nc()
    nc = _CACHE["nc"]

    in_maps = _build_in_maps(x, qkv_w, qkv_b, out_w)
    out_b = np.asarray(out_b, dtype=np.float32)

    try:
        res = run_bass_kernel_spmd(nc, in_maps, core_ids=list(range(N_CORES)))
    except ModuleNotFoundError:
        # BASS_TRACE set but the NTFF profile hook isn't importable here
        import os
        os.environ["BASS_NEVER_TRACE"] = "1"
        res = run_bass_kernel_spmd(nc, in_maps, core_ids=list(range(N_CORES)))

    out = np.empty((B, T, C), dtype=np.float32)
    for b in range(B):
        pt = (res.results[2 * b]["ot"].astype(np.float32)
              + res.results[2 * b + 1]["ot"].astype(np.float32))  # (C, T)
        out[b] = pt.T + out_b[None, :]
    return out


# revision 36
# speedup vs baseline: 1.1105x; 1.0471x over previous
"""Causal self-attention (B=4, T=2048, C=1024, H=16, D=64) on 8 TRN2 NeuronCores.

Sharding: core = (batch b, head-group g) with b = core // 2, g = core % 2.
Each core computes heads [8g, 8g+8) of batch b and produces the partial
out-projection (C, T) fp16 for its head group; the host sums the two
head-group partials per batch and adds the output bias.

Speed structure (vs the all-fp16 baseline):
- Projections and scores run as fp8e4 DoubleRow matmuls (0.5 cycles/row,
  two k-tiles per pass). Weight accuracy is restored with a hi/lo split
  (hi = fp8(S*w), lo = fp8(S*w - hi)); weights are prescaled (SQ=32 for
  qkv, SV=8 for V) to clear fp8e4's 2^-9 subnormal floor. SQ is undone
  in the rope cos/sin tables; SV cancels in softmax normalization since
  the denominator ones-column carries the same scale.
- Scores use a "zero slot": lhsT k-group 1 points at a zeroed column
  block of the rq/rk tile, so a K=64 fp16-shaped matmul still gets the
  DoubleRow rate. q/k are cast to fp8 by the rope add.
- att@V: exp() writes fp8 straight from the Activation engine; V is
  stored as interleaved (v_hi, v_lo) fp8 pairs and one DoubleRow matmul
  computes v_hi^T e + v_lo^T e per s-chunk via a stride-0 broadcast of e.
  Diagonal s-chunks (which carry the concentrated softmax weight) use an
  fp16 e and an fp16 V copy instead - that one change cuts the output
  error ~2x while costing only the extra diagonal columns.
- Phases are emission-interleaved so the PE/DVE work of the V projection,
  next chunk's qk+rope, and the output projection hide under the
  Activation-engine-bound attention inner loop.
"""

import numpy as np

B, T, C = 4, 2048, 1024
H, D = 16, 64
N_CORES = 8
HPG = H // 2            # heads per core (group)
NCHUNK = 4              # head-pair chunks per core
KT = 8                  # k-tiles of 128 over C
TT = 4                  # t-tiles of 512 over T
NT = 512                # t tile (matmul N)
VS = 66                 # v column stride per head (64 dims + ones + pad)
VW = HPG * VS           # 528 v columns per k-chunk block
ROPE_BASE = 10000.0
SQ = 32.0               # qk weight prescale (undone via cs/css tables)
SV = 8.0                # v weight prescale (cancels in softmax normalization)

_CACHE = {}


def _build_nc():
    import concourse.bass as bass
    import concourse.tile as tile
    from concourse import bacc, mybir
    from contextlib import ExitStack

    f16 = mybir.dt.float16
    f32 = mybir.dt.float32
    f8 = mybir.dt.float8e4
    DR = mybir.MatmulPerfMode.DoubleRow

    nc = bacc.Bacc(
        "TRN2",
        target_bir_lowering=False,
        debug=False,
        enable_asserts=True,
        num_devices=N_CORES,
    )

    x8h_d = nc.dram_tensor("x8h", (10 * 128, T), f8, kind="ExternalInput").ap()
    x8l_d = nc.dram_tensor("x8l", (KT * 128, T), f8, kind="ExternalInput").ap()
    wqk8h_d = nc.dram_tensor("wqk8h", (128, 10 * 1024), f8, kind="ExternalInput").ap()
    wv8h_d = nc.dram_tensor("wv8h", (128, 10 * VW), f8, kind="ExternalInput").ap()
    wv8l_d = nc.dram_tensor("wv8l", (128, KT * VW), f8, kind="ExternalInput").ap()
    wo_d = nc.dram_tensor("wo", (128, NCHUNK * 1024), f16, kind="ExternalInput").ap()
    cs_d = nc.dram_tensor("cs", (128, T), f16, kind="ExternalInput").ap()
    css_d = nc.dram_tensor("css", (128, T), f16, kind="ExternalInput").ap()
    ot_d = nc.dram_tensor("ot", (1024, T), f16, kind="ExternalOutput").ap()

    SHUF = list(range(16, 32)) + list(range(0, 16))

    def slot_b(ap, w):
        # [K, 2, w] view with a stride-0 k-group dim (broadcast the same block)
        return bass.AP(ap.tensor, ap.offset, [list(ap.ap[0]), [0, 2], [1, w]])

    with tile.TileContext(nc) as tc:
        with ExitStack() as ctx, nc.allow_low_precision("fp8 attention pipeline"):
            consts = ctx.enter_context(tc.tile_pool(name="consts", bufs=1))
            qk_pool = ctx.enter_context(tc.tile_pool(name="qk", bufs=2))
            rtmp = ctx.enter_context(tc.tile_pool(name="rtmp", bufs=4))
            e8_pool = ctx.enter_context(tc.tile_pool(name="e8", bufs=6))
            e16_pool = ctx.enter_context(tc.tile_pool(name="e16", bufs=4))
            small = ctx.enter_context(tc.tile_pool(name="small", bufs=3))
            osb = ctx.enter_context(tc.tile_pool(name="osb", bufs=6))
            ps_big = ctx.enter_context(tc.tile_pool(name="psbig", bufs=2, space="PSUM"))
            ps_s = ctx.enter_context(tc.tile_pool(name="pss", bufs=2, space="PSUM"))
            ps_y = ctx.enter_context(tc.tile_pool(name="psy", bufs=1, space="PSUM"))

            # ---- resident tiles + input DMA ----
            # issue order follows first use: qk weights + x (t-half 0) feed the
            # head rope chain, then wv for vproj, then the rest
            x8h = consts.tile([128, 10 * T], f8)
            x8l = consts.tile([128, KT * T], f8)
            wqk8h = consts.tile([128, 10 * 1024], f8)
            wv8h = consts.tile([128, 10 * VW], f8)
            wv8l = consts.tile([128, KT * VW], f8)
            cs = consts.tile([128, T], f16)
            css = consts.tile([128, T], f16)
            wo = consts.tile([128, NCHUNK * 1024], f16)

            def dma_split(dst, src, width, parts):
                step = width // parts
                for i in range(parts):
                    nc.sync.dma_start(dst[:, i * step:(i + 1) * step],
                                      src[:, i * step:(i + 1) * step])

            def x_half(half):
                t0, t1 = half * (T // 2), (half + 1) * (T // 2)
                for kc in range(10):
                    nc.sync.dma_start(x8h[:, kc * T + t0: kc * T + t1],
                                      x8h_d[kc * 128:(kc + 1) * 128, t0:t1])
                for kc in range(KT):
                    nc.sync.dma_start(x8l[:, kc * T + t0: kc * T + t1],
                                      x8l_d[kc * 128:(kc + 1) * 128, t0:t1])

            dma_split(wqk8h, wqk8h_d, 10 * 1024, 5)
            x_half(0)
            dma_split(cs, cs_d, T, 2)
            dma_split(css, css_d, T, 2)
            dma_split(wv8h, wv8h_d, 10 * VW, 2)
            dma_split(wv8l, wv8l_d, KT * VW, 2)
            x_half(1)
            dma_split(wo, wo_d, NCHUNK * 1024, 4)

            v8 = consts.tile([128, 16 * 2 * VW], f8)    # (m, hi|lo, col)
            v16 = consts.tile([128, 16 * VW], f16)      # fp16 V for diagonal chunks
            y_all = consts.tile([128, NCHUNK * T], f16)

            x8h3 = x8h[:].rearrange("p (kc t) -> p kc t", kc=10)
            x8l3 = x8l[:].rearrange("p (kc t) -> p kc t", kc=KT)
            wv8h3 = wv8h[:].rearrange("p (kc c) -> p kc c", kc=10)
            wv8l3 = wv8l[:].rearrange("p (kc c) -> p kc c", kc=KT)
            wqk8h3 = wqk8h[:].rearrange("p (kc c) -> p kc c", kc=10)
            v84 = v8[:].rearrange("p (m s c) -> p m s c", m=16, s=2)

            # ---- emission helpers (phases are interleaved for overlap) ----
            HVW = VW // 2   # 264: v columns for 4 heads incl. their ones cols

            def emit_vproj_m(m):
                # two half-width PSUM tiles split at the head-4 boundary so
                # vproj never touches the scores pool (ps_s)
                psa0 = ps_big.tile([128, HVW], f32, tag="big")
                psa1 = ps_big.tile([128, HVW], f32, tag="big")
                psa = (psa0, psa1)
                mm = []
                for j in range(KT // 2):
                    kk = slice(2 * j, 2 * j + 2)
                    mm.append((x8h3[:, kk, m * 128:(m + 1) * 128], wv8h3[:, kk, :]))
                    mm.append((x8l3[:, kk, m * 128:(m + 1) * 128], wv8h3[:, kk, :]))
                    mm.append((x8h3[:, kk, m * 128:(m + 1) * 128], wv8l3[:, kk, :]))
                mm.append((x8h3[:, 8:10, m * 128:(m + 1) * 128], wv8h3[:, 8:10, :]))
                n = len(mm)
                for i, (lhsT, rhs) in enumerate(mm):
                    for half in range(2):
                        nc.tensor.matmul(psa[half][:], lhsT,
                                         rhs[:, :, half * HVW:(half + 1) * HVW],
                                         start=(i == 0), stop=(i == n - 1),
                                         perf_mode=DR)
                base = m * 2 * VW
                for half in range(2):
                    o = half * HVW
                    nc.vector.tensor_copy(v8[:, base + o: base + o + HVW], psa[half][:])
                    nc.vector.tensor_copy(v16[:, m * VW + o: m * VW + o + HVW],
                                          psa[half][:])
                    nc.vector.tensor_tensor(
                        out=v8[:, base + VW + o: base + VW + o + HVW],
                        in0=psa[half][:], in1=v8[:, base + o: base + o + HVW],
                        op=mybir.AluOpType.subtract)

            qk_tiles = {}

            def ensure_qk(c):
                if c not in qk_tiles:
                    rq = qk_pool.tile([128, T + 128], f8, tag="rq")
                    rk = qk_pool.tile([128, T + 128], f8, tag="rk")
                    nc.gpsimd.memset(rq[:, T:T + 128], 0.0)
                    nc.gpsimd.memset(rk[:, T:T + 128], 0.0)
                    qk_tiles[c] = (rq, rk)
                return qk_tiles[c]

            def emit_rope_a(c, tt, which):
                """Projection + shuffle + the two rope muls (x2 on gpsimd)."""
                dst = ensure_qk(c)[which]
                t0 = tt * NT
                cm = c * 256 + which * 128
                ps = ps_big.tile([128, 512], f32, tag="big")
                mm = []
                for j in range(KT // 2):
                    kk = slice(2 * j, 2 * j + 2)
                    mm.append((wqk8h3[:, kk, cm:cm + 128], x8h3[:, kk, t0:t0 + NT]))
                    mm.append((wqk8h3[:, kk, cm:cm + 128], x8l3[:, kk, t0:t0 + NT]))
                mm.append((wqk8h3[:, 8:10, cm:cm + 128], x8h3[:, 8:10, t0:t0 + NT]))
                n = len(mm)
                for i, (lhsT, rhs) in enumerate(mm):
                    nc.tensor.matmul(ps[:], lhsT, rhs,
                                     start=(i == 0), stop=(i == n - 1), perf_mode=DR)
                s_t = rtmp.tile([128, 512], f32, tag="s")
                nc.vector.stream_shuffle(s_t[:], ps[:], SHUF)
                x1 = rtmp.tile([128, 512], f16, tag="x1")
                nc.vector.tensor_mul(x1[:], ps[:], cs[:, t0:t0 + NT])
                x2 = rtmp.tile([128, 512], f16, tag="x2")
                nc.gpsimd.tensor_mul(x2[:], s_t[:], css[:, t0:t0 + NT])
                return x1, x2

            def emit_rope_b(c, tt, which, x1, x2):
                """Deferred fp8 add - emitted one instance behind stage a so
                the gpsimd x2 mul never stalls the DVE queue."""
                dst = ensure_qk(c)[which]
                t0 = tt * NT
                nc.vector.tensor_add(dst[:, t0:t0 + NT], x1[:], x2[:])

            def rope_items(c):
                """Pending-queue closures for all 8 rope instances of chunk c,
                with each add deferred two slots behind its producer."""
                items = []
                for tt in range(TT):
                    st = {}
                    for which in (0, 1):
                        def a_fn(c=c, tt=tt, which=which, st=st):
                            st[which] = emit_rope_a(c, tt, which)
                        items.append(a_fn)
                    for which in (0, 1):
                        def b_fn(c=c, tt=tt, which=which, st=st):
                            emit_rope_b(c, tt, which, *st[which])
                        items.append(b_fn)
                return items

            pending = []

            def emit_scores(c, tt, sc):
                rq, rk = qk_tiles[c]
                t0 = tt * NT
                s0 = sc * 128
                dlt = max(0, s0 - t0)
                w = NT - dlt
                sp = ps_s.tile([128, 1024], f32, tag="s")
                for h in range(2):
                    sl = rk[h * 64:(h + 1) * 64, s0:s0 + 128]
                    lhsT = bass.AP(sl.tensor, sl.offset,
                                   [list(sl.ap[0]), [T - s0, 2], [1, 128]])
                    sr = rq[h * 64:(h + 1) * 64, t0 + dlt:t0 + NT]
                    rhs = bass.AP(sr.tensor, sr.offset,
                                  [list(sr.ap[0]), [0, 2], [1, w]])
                    nc.tensor.matmul(sp[:, h * NT + dlt:(h + 1) * NT], lhsT, rhs,
                                     start=True, stop=True, perf_mode=DR,
                                     tile_position=(h * 64, 0))
                return sp

            def emit_exp(tt, sc, sp):
                t0 = tt * NT
                s0 = sc * 128
                dlt = max(0, s0 - t0)
                diag = s0 + 127 > t0
                e_t = (e16_pool if diag else e8_pool).tile(
                    [128, 1024], f16 if diag else f8)
                s3 = sp[:].rearrange("p (a b) -> p a b", a=2)[:, :, dlt:]
                e3 = e_t[:].rearrange("p (a b) -> p a b", a=2)[:, :, dlt:]
                nc.scalar.activation(e3, s3, mybir.ActivationFunctionType.Exp,
                                     bias=0.0, scale=0.125)
                if diag:
                    # keep iff j' >= p; for j' >= 128 that's always true,
                    # so only the first 128 columns need the select
                    e3m = e_t[:].rearrange("p (a b) -> p a b", a=2)[:, :, dlt:dlt + 128]
                    nc.gpsimd.affine_select(
                        out=e3m, in_=e3m,
                        compare_op=mybir.AluOpType.is_ge,
                        fill=0.0, base=0,
                        pattern=[[0, 2], [1, 128]], channel_multiplier=-1)
                return e_t

            def emit_attv(c, tt, sc, e_t, yp):
                t0 = tt * NT
                sc_max = (t0 + NT) // 128
                s0 = sc * 128
                dlt = max(0, s0 - t0)
                w = NT - dlt
                diag = s0 + 127 > t0
                for h in range(2):
                    vc = VS * (2 * c + h)
                    if diag:
                        nc.tensor.matmul(
                            yp[:, h * NT + dlt:(h + 1) * NT],
                            v16[:, sc * VW + vc: sc * VW + vc + 65],
                            e_t[:, h * NT + dlt:(h + 1) * NT],
                            start=(sc == 0), stop=(sc == sc_max - 1),
                            skip_group_check=True)
                    else:
                        nc.tensor.matmul(
                            yp[:, h * NT + dlt:(h + 1) * NT],
                            v84[:, sc, :, vc: vc + 65],
                            slot_b(e_t[:, h * NT + dlt: (h + 1) * NT], w),
                            start=(sc == 0), stop=(sc == sc_max - 1),
                            perf_mode=DR, skip_group_check=True)

            def emit_norm(c, tt, yp):
                t0 = tt * NT
                rd = small.tile([1, 1024], f16, tag="rd")
                nc.vector.reciprocal(rd[:], yp[64:65, :])
                rbc = small.tile([64, 1024], f16, tag="rbc")
                nc.gpsimd.partition_broadcast(rbc[:], rd[:])
                for h in range(2):
                    nc.vector.tensor_mul(
                        y_all[h * 64:(h + 1) * 64, c * T + t0: c * T + t0 + NT],
                        yp[0:64, h * NT:(h + 1) * NT],
                        rbc[:, h * NT:(h + 1) * NT])

            def emit_attn_chunk(c, on_norm=None):
                """Software-pipelined attention stream for one chunk: scores
                run one s-chunk ahead of att@V so the PE's in-order queue
                never starves the Activation engine's exp pipeline."""
                seq = [(tt, sc) for tt in range(TT)
                       for sc in range((tt + 1) * NT // 128)]
                sps = {}
                ets = {}
                yps = {}
                sps[0] = emit_scores(c, *seq[0])

                def attv_and_norm(j):
                    tt_j, sc_j = seq[j]
                    if sc_j == 0:
                        yp_t = ps_y.tile([65, 1024], f32, tag="y")
                        yps[tt_j] = yp_t
                    emit_attv(c, tt_j, sc_j, ets.pop(j), yps[tt_j])
                    if sc_j == (tt_j + 1) * NT // 128 - 1:
                        emit_norm(c, tt_j, yps.pop(tt_j))
                        if on_norm is not None:
                            on_norm(tt_j)

                for i, (tt, sc) in enumerate(seq):
                    if i + 1 < len(seq):
                        sp_t = emit_scores(c, *seq[i + 1])
                        sps[i + 1] = sp_t
                    ets[i] = emit_exp(tt, sc, sps.pop(i))
                    if i >= 1:
                        attv_and_norm(i - 1)  # att@V one s-chunk behind exp
                    if pending:
                        pending.pop(0)()
                for i in (len(seq) - 1,):
                    attv_and_norm(i)

            def emit_oproj_ct(tt, ct):
                t0 = tt * NT
                po = ps_big.tile([128, 512], f32, tag="big")
                for c in range(NCHUNK):
                    nc.tensor.matmul(po[:], wo[:, c * 1024 + ct * 128: c * 1024 + ct * 128 + 128],
                                     y_all[:, c * T + t0: c * T + t0 + NT],
                                     start=(c == 0), stop=(c == NCHUNK - 1))
                ob = osb.tile([128, 512], f16)
                nc.vector.tensor_copy(ob[:], po[:])
                nc.sync.dma_start(ot_d[ct * 128:(ct + 1) * 128, t0:t0 + NT], ob[:])

            # ---- head phase: chunk-0 rope tt0 first, then first v-slices ----
            with nc.named_scope("head"):
                st = {}
                for which in (0, 1):
                    st[which] = emit_rope_a(0, 0, which)
                for which in (0, 1):
                    emit_rope_b(0, 0, which, *st[which])
                for m in range(4):
                    emit_vproj_m(m)
                st = {}
                for which in (0, 1):
                    st[which] = emit_rope_a(0, 1, which)
                for which in (0, 1):
                    emit_rope_b(0, 1, which, *st[which])

            # ---- pending helper work, drained one item per s-chunk of the
            # Act-bound attention inner loop (deadlines commented) ----
            rope0 = rope_items(0)[8:]   # chunk-0 tt2/tt3 (8 items)
            rope1 = rope_items(1)
            # c0 queue: vproj m4..m15 + rope(c0,tt2/tt3) + rope(c1).
            # m_k is popped >= (k-4) slots in; attV(c0) first reads m_k at
            # global slot >= k (4*tt slots precede the tt that reads it), and
            # rope(c0,tt2) B-items sit at index <= 11 < 12 slots before tt2.
            pending.extend([lambda m=m: emit_vproj_m(m) for m in (4, 5)])
            pending.extend(rope0[0:2])
            pending.extend([lambda m=m: emit_vproj_m(m) for m in (6, 7)])
            pending.extend(rope0[2:4])
            pending.extend([lambda m=m: emit_vproj_m(m) for m in (8, 9)])
            pending.extend(rope0[4:6])
            pending.extend([lambda m=m: emit_vproj_m(m) for m in (10, 11)])
            pending.extend(rope0[6:8])
            pending.extend([lambda m=m: emit_vproj_m(m) for m in (12, 13, 14, 15)])
            pending.extend(rope1)

            def queue_oproj(tt):
                if tt < 3:  # tt3 runs in the tail
                    pending.extend(
                        [lambda ct=ct, tt=tt: emit_oproj_ct(tt, ct)
                         for ct in range(8)])

            for c in range(NCHUNK):
                with nc.named_scope(f"attn{c}"):
                    if c in (1, 2):
                        pending.extend(rope_items(c + 1))
                    emit_attn_chunk(
                        c, on_norm=queue_oproj if c == NCHUNK - 1 else None)

            with nc.named_scope("tail"):
                while pending:
                    pending.pop(0)()
                for ct in range(8):
                    emit_oproj_ct(3, ct)

    nc.compile()
    return nc


def _fp8_split(a, np8, scale=1.0):
    """scale*a -> (hi, lo) fp8 with hi + lo ~= scale*a to ~0.13%.

    The scale lifts 0.02-magnitude weights above fp8e4's 2^-9 subnormal
    floor so the lo residual can actually represent the hi rounding error.
    """
    a = np.asarray(a, dtype=np.float32) * scale
    hi = a.astype(np8)
    lo = (a - hi.astype(np.float32)).astype(np8)
    return hi, lo


def _prep_inputs(x, qkv_w, qkv_b):
    """Build the per-core input maps (all host-side numpy)."""
    from concourse import mybir
    np8 = mybir.dt.np(mybir.dt.float8e4)

    x = np.asarray(x, dtype=np.float32)
    qkv_w = np.asarray(qkv_w, dtype=np.float32)
    qkv_b = np.asarray(qkv_b, dtype=np.float32)

    # x8 per batch: hi [10*128, T] (ktiles 8/9 = ones row), lo [8*128, T]
    x8hs, x8ls = [], []
    for b in range(B):
        xh = np.zeros((10 * 128, T), dtype=np8)
        xl = np.zeros((KT * 128, T), dtype=np8)
        hi, lo = _fp8_split(x[b].T, np8)
        xh[:C] = hi
        xl[:C] = lo
        xh[C] = np8(1.0)        # aug ktile 8: ones row
        xh[9 * 128] = np8(1.0)  # aug ktile 9: duplicate ones row
        x8hs.append(xh)
        x8ls.append(xl)

    r = np.arange(64)
    d_r = 2 * ((r // 32) * 16 + (r % 16)) + ((r % 32) >= 16)  # row -> head dim
    p = np.arange(128)
    f_p = ((p // 32) % 2) * 16 + (p % 16)

    ins_g = []
    for g in range(2):
        # wqk8h: [p, kc*1024 + c*256 + which*128 + m]; kc8/9 = bias hi/lo on row 0
        wqkh = np.zeros((128, 10 * 1024), dtype=np8)
        for c in range(NCHUNK):
            for which in range(2):  # 0=q, 1=k
                rows = np.concatenate([
                    which * C + (8 * g + 2 * c + hh) * 64 + d_r for hh in range(2)
                ])  # 128 feature rows
                blk = qkv_w[rows, :]          # (128 feat, 1024 k)
                cm = c * 256 + which * 128
                for kc in range(KT):
                    hi, _lo = _fp8_split(blk[:, kc * 128:(kc + 1) * 128].T, np8, SQ)
                    wqkh[:, kc * 1024 + cm: kc * 1024 + cm + 128] = hi
                bh, bl = _fp8_split(qkv_b[rows], np8, SQ)
                wqkh[0, 8 * 1024 + cm: 8 * 1024 + cm + 128] = bh
                wqkh[0, 9 * 1024 + cm: 9 * 1024 + cm + 128] = bl
        # wv8: [p, kc*VW + VS*h + j]; kc8 = aug hi (bias+ones), kc9 = aug lo (bias)
        wva = np.zeros((KT * 128, VW), dtype=np.float32)
        aug = np.zeros((128, VW), dtype=np.float32)
        for h in range(HPG):
            rows = 2 * C + (8 * g + h) * 64 + np.arange(64)
            wva[:, VS * h: VS * h + 64] = qkv_w[rows, :].T
            aug[0, VS * h: VS * h + 64] = qkv_b[rows]
            aug[0, VS * h + 64] = 1.0
        wvh = np.zeros((128, 10 * VW), dtype=np8)
        wvl = np.zeros((128, KT * VW), dtype=np8)
        for kc in range(KT):
            hi, lo = _fp8_split(wva[kc * 128:(kc + 1) * 128], np8, SV)
            wvh[:, kc * VW:(kc + 1) * VW] = hi
            wvl[:, kc * VW:(kc + 1) * VW] = lo
        augh, augl = _fp8_split(aug, np8, SV)  # ones col becomes SV (exact in fp8)
        augl[0, VS * np.arange(HPG) + 64] = np8(0.0)  # ones col only in hi
        wvh[:, 8 * VW: 9 * VW] = augh
        wvh[:, 9 * VW: 10 * VW] = augl
        ins_g.append((wqkh, wvh, wvl))

    # rope tables (divided by SQ to undo the qk weight prescale)
    inv_freq = (1.0 / (ROPE_BASE ** (np.arange(0, D, 2) / D))).astype(np.float64)
    t = np.arange(T, dtype=np.float64)
    ang = t[None, :] * inv_freq[f_p][:, None]          # (128, T)
    cs = (np.cos(ang) / SQ).astype(np.float16)
    sgn = np.where((p % 32) < 16, -1.0, 1.0)[:, None]
    css = (sgn * np.sin(ang) / SQ).astype(np.float16)

    return x8hs, x8ls, ins_g, cs, css


def _prep_wo(out_w, g):
    out_w = np.asarray(out_w, dtype=np.float32)
    wo = np.empty((128, NCHUNK * 1024), dtype=np.float16)
    for c in range(NCHUNK):
        rows = np.concatenate([(8 * g + 2 * c + hh) * 64 + np.arange(64) for hh in range(2)])
        wo[:, c * 1024:(c + 1) * 1024] = out_w[:, rows].astype(np.float16).T
    return wo


def _build_in_maps(x, qkv_w, qkv_b, out_w):
    x8hs, x8ls, ins_g, cs, css = _prep_inputs(x, qkv_w, qkv_b)
    wos = [_prep_wo(out_w, g) for g in range(2)]
    in_maps = []
    for core in range(N_CORES):
        b, g = core // 2, core % 2
        wqkh, wvh, wvl = ins_g[g]
        in_maps.append({
            "x8h": x8hs[b], "x8l": x8ls[b],
            "wqk8h": wqkh,
            "wv8h": wvh, "wv8l": wvl,
            "wo": wos[g], "cs": cs, "css": css,
        })
    return in_maps


def kernel(x, qkv_w, qkv_b, out_w, out_b):
    from concourse.bass_utils import run_bass_kernel_spmd

    if "nc" not in _CACHE:
        _CACHE["nc"] = _build_nc()
    nc = _CACHE["nc"]

    in_maps = _build_in_maps(x, qkv_w, qkv_b, out_w)
    out_b = np.asarray(out_b, dtype=np.float32)

    try:
        res = run_bass_kernel_spmd(nc, in_maps, core_ids=list(range(N_CORES)))
    except ModuleNotFoundError:
        # BASS_TRACE set but the NTFF profile hook isn't importable here
        import os
        os.environ["BASS_NEVER_TRACE"] = "1"
        res = run_bass_kernel_spmd(nc, in_maps, core_ids=list(range(N_CORES)))

    out = np.empty((B, T, C), dtype=np.float32)
    for b in range(B):
        pt = (res.results[2 * b]["ot"].astype(np.float32)
              + res.results[2 * b + 1]["ot"].astype(np.float32))  # (C, T)
        out[b] = pt.T + out_b[None, :]
    return out


# revision 39
# speedup vs baseline: 1.1375x; 1.0243x over previous
"""Causal self-attention (B=4, T=2048, C=1024, H=16, D=64) on 8 TRN2 NeuronCores.

Sharding: core = (batch b, head-group g) with b = core // 2, g = core % 2.
Each core computes heads [8g, 8g+8) of batch b and produces the partial
out-projection (C, T) fp16 for its head group; the host sums the two
head-group partials per batch and adds the output bias.

Speed structure (vs the all-fp16 baseline):
- Projections and scores run as fp8e4 DoubleRow matmuls (0.5 cycles/row,
  two k-tiles per pass). Weight accuracy is restored with a hi/lo split
  (hi = fp8(S*w), lo = fp8(S*w - hi)); weights are prescaled (SQ=32 for
  qkv, SV=8 for V) to clear fp8e4's 2^-9 subnormal floor. SQ is undone
  in the rope cos/sin tables; SV cancels in softmax normalization since
  the denominator ones-column carries the same scale.
- Scores use a "zero slot": lhsT k-group 1 points at a zeroed column
  block of the rq/rk tile, so a K=64 fp16-shaped matmul still gets the
  DoubleRow rate. q/k are cast to fp8 by the rope add.
- att@V: exp() writes fp8 straight from the Activation engine; V is
  stored as interleaved (v_hi, v_lo) fp8 pairs and one DoubleRow matmul
  computes v_hi^T e + v_lo^T e per s-chunk via a stride-0 broadcast of e.
  Diagonal s-chunks (which carry the concentrated softmax weight) use an
  fp16 e and an fp16 V copy instead - that one change cuts the output
  error ~2x while costing only the extra diagonal columns.
- Phases are emission-interleaved so the PE/DVE work of the V projection,
  next chunk's qk+rope, and the output projection hide under the
  Activation-engine-bound attention inner loop.
"""

import numpy as np

B, T, C = 4, 2048, 1024
H, D = 16, 64
N_CORES = 8
HPG = H // 2            # heads per core (group)
NCHUNK = 4              # head-pair chunks per core
KT = 8                  # k-tiles of 128 over C
TT = 4                  # t-tiles of 512 over T
NT = 512                # t tile (matmul N)
VS = 66                 # v column stride per head (64 dims + ones + pad)
VW = HPG * VS           # 528 v columns per k-chunk block
ROPE_BASE = 10000.0
SQ = 32.0               # qk weight prescale (undone via cs/css tables)
SV = 8.0                # v weight prescale (cancels in softmax normalization)

_CACHE = {}


def _build_nc():
    import concourse.bass as bass
    import concourse.tile as tile
    from concourse import bacc, mybir
    from contextlib import ExitStack

    f16 = mybir.dt.float16
    f32 = mybir.dt.float32
    f8 = mybir.dt.float8e4
    DR = mybir.MatmulPerfMode.DoubleRow

    nc = bacc.Bacc(
        "TRN2",
        target_bir_lowering=False,
        debug=False,
        enable_asserts=True,
        num_devices=N_CORES,
    )

    x8h_d = nc.dram_tensor("x8h", (10 * 128, T), f8, kind="ExternalInput").ap()
    x8l_d = nc.dram_tensor("x8l", (KT * 128, T), f8, kind="ExternalInput").ap()
    wqk8h_d = nc.dram_tensor("wqk8h", (128, 10 * 1024), f8, kind="ExternalInput").ap()
    wv8h_d = nc.dram_tensor("wv8h", (128, 10 * VW), f8, kind="ExternalInput").ap()
    wv8l_d = nc.dram_tensor("wv8l", (128, KT * VW), f8, kind="ExternalInput").ap()
    wo_d = nc.dram_tensor("wo", (128, NCHUNK * 1024), f16, kind="ExternalInput").ap()
    cs_d = nc.dram_tensor("cs", (128, T), f16, kind="ExternalInput").ap()
    css_d = nc.dram_tensor("css", (128, T), f16, kind="ExternalInput").ap()
    ot_d = nc.dram_tensor("ot", (1024, T), f16, kind="ExternalOutput").ap()

    SHUF = list(range(16, 32)) + list(range(0, 16))

    def slot_b(ap, w):
        # [K, 2, w] view with a stride-0 k-group dim (broadcast the same block)
        return bass.AP(ap.tensor, ap.offset, [list(ap.ap[0]), [0, 2], [1, w]])

    with tile.TileContext(nc) as tc:
        with ExitStack() as ctx, nc.allow_low_precision("fp8 attention pipeline"):
            consts = ctx.enter_context(tc.tile_pool(name="consts", bufs=1))
            qk_pool = ctx.enter_context(tc.tile_pool(name="qk", bufs=2))
            rtmp = ctx.enter_context(tc.tile_pool(name="rtmp", bufs=4))
            e8_pool = ctx.enter_context(tc.tile_pool(name="e8", bufs=8))
            e16_pool = ctx.enter_context(tc.tile_pool(name="e16", bufs=6))
            small = ctx.enter_context(tc.tile_pool(name="small", bufs=3))
            osb = ctx.enter_context(tc.tile_pool(name="osb", bufs=6))
            ps_big = ctx.enter_context(tc.tile_pool(name="psbig", bufs=2, space="PSUM"))
            ps_s = ctx.enter_context(tc.tile_pool(name="pss", bufs=2, space="PSUM"))
            ps_y = ctx.enter_context(tc.tile_pool(name="psy", bufs=1, space="PSUM"))

            # ---- resident tiles + input DMA ----
            # issue order follows first use: qk weights + x (t-half 0) feed the
            # head rope chain, then wv for vproj, then the rest
            x8h = consts.tile([128, 10 * T], f8)
            x8l = consts.tile([128, KT * T], f8)
            wqk8h = consts.tile([128, 10 * 1024], f8)
            wv8h = consts.tile([128, 10 * VW], f8)
            wv8l = consts.tile([128, KT * VW], f8)
            cs = consts.tile([128, T], f16)
            css = consts.tile([128, T], f16)
            wo = consts.tile([128, NCHUNK * 1024], f16)

            def dma_split(dst, src, width, parts):
                step = width // parts
                for i in range(parts):
                    nc.sync.dma_start(dst[:, i * step:(i + 1) * step],
                                      src[:, i * step:(i + 1) * step])

            def x_half(half):
                t0, t1 = half * (T // 2), (half + 1) * (T // 2)
                for kc in range(10):
                    nc.sync.dma_start(x8h[:, kc * T + t0: kc * T + t1],
                                      x8h_d[kc * 128:(kc + 1) * 128, t0:t1])
                for kc in range(KT):
                    nc.sync.dma_start(x8l[:, kc * T + t0: kc * T + t1],
                                      x8l_d[kc * 128:(kc + 1) * 128, t0:t1])

            dma_split(wqk8h, wqk8h_d, 10 * 1024, 5)
            x_half(0)
            dma_split(cs, cs_d, T, 2)
            dma_split(css, css_d, T, 2)
            dma_split(wv8h, wv8h_d, 10 * VW, 2)
            dma_split(wv8l, wv8l_d, KT * VW, 2)
            x_half(1)
            dma_split(wo, wo_d, NCHUNK * 1024, 4)

            v8 = consts.tile([128, 16 * 2 * VW], f8)    # (m, hi|lo, col)
            y_all = consts.tile([128, NCHUNK * T], f16)

            x8h3 = x8h[:].rearrange("p (kc t) -> p kc t", kc=10)
            x8l3 = x8l[:].rearrange("p (kc t) -> p kc t", kc=KT)
            wv8h3 = wv8h[:].rearrange("p (kc c) -> p kc c", kc=10)
            wv8l3 = wv8l[:].rearrange("p (kc c) -> p kc c", kc=KT)
            wqk8h3 = wqk8h[:].rearrange("p (kc c) -> p kc c", kc=10)
            v84 = v8[:].rearrange("p (m s c) -> p m s c", m=16, s=2)

            # ---- emission helpers (phases are interleaved for overlap) ----
            HVW = VW // 2   # 264: v columns for 4 heads incl. their ones cols

            def emit_vproj_m(m):
                # two half-width PSUM tiles split at the head-4 boundary so
                # vproj never touches the scores pool (ps_s)
                psa0 = ps_big.tile([128, HVW], f32, tag="big")
                psa1 = ps_big.tile([128, HVW], f32, tag="big")
                psa = (psa0, psa1)
                mm = []
                for j in range(KT // 2):
                    kk = slice(2 * j, 2 * j + 2)
                    mm.append((x8h3[:, kk, m * 128:(m + 1) * 128], wv8h3[:, kk, :]))
                    mm.append((x8l3[:, kk, m * 128:(m + 1) * 128], wv8h3[:, kk, :]))
                    mm.append((x8h3[:, kk, m * 128:(m + 1) * 128], wv8l3[:, kk, :]))
                mm.append((x8h3[:, 8:10, m * 128:(m + 1) * 128], wv8h3[:, 8:10, :]))
                n = len(mm)
                for i, (lhsT, rhs) in enumerate(mm):
                    for half in range(2):
                        nc.tensor.matmul(psa[half][:], lhsT,
                                         rhs[:, :, half * HVW:(half + 1) * HVW],
                                         start=(i == 0), stop=(i == n - 1),
                                         perf_mode=DR)
                base = m * 2 * VW
                for half in range(2):
                    o = half * HVW
                    nc.vector.tensor_copy(v8[:, base + o: base + o + HVW], psa[half][:])
                    nc.vector.tensor_tensor(
                        out=v8[:, base + VW + o: base + VW + o + HVW],
                        in0=psa[half][:], in1=v8[:, base + o: base + o + HVW],
                        op=mybir.AluOpType.subtract)

            qk_tiles = {}

            def ensure_qk(c):
                if c not in qk_tiles:
                    rq = qk_pool.tile([128, T + 128], f8, tag="rq")
                    rk = qk_pool.tile([128, T + 128], f8, tag="rk")
                    nc.gpsimd.memset(rq[:, T:T + 128], 0.0)
                    nc.gpsimd.memset(rk[:, T:T + 128], 0.0)
                    qk_tiles[c] = (rq, rk)
                return qk_tiles[c]

            def emit_rope_a(c, tt, which):
                """Projection + shuffle + the two rope muls (x2 on gpsimd)."""
                dst = ensure_qk(c)[which]
                t0 = tt * NT
                cm = c * 256 + which * 128
                ps = ps_big.tile([128, 512], f32, tag="big")
                mm = []
                for j in range(KT // 2):
                    kk = slice(2 * j, 2 * j + 2)
                    mm.append((wqk8h3[:, kk, cm:cm + 128], x8h3[:, kk, t0:t0 + NT]))
                    mm.append((wqk8h3[:, kk, cm:cm + 128], x8l3[:, kk, t0:t0 + NT]))
                mm.append((wqk8h3[:, 8:10, cm:cm + 128], x8h3[:, 8:10, t0:t0 + NT]))
                n = len(mm)
                for i, (lhsT, rhs) in enumerate(mm):
                    nc.tensor.matmul(ps[:], lhsT, rhs,
                                     start=(i == 0), stop=(i == n - 1), perf_mode=DR)
                s_t = rtmp.tile([128, 512], f32, tag="s")
                nc.vector.stream_shuffle(s_t[:], ps[:], SHUF)
                x1 = rtmp.tile([128, 512], f16, tag="x1")
                nc.vector.tensor_mul(x1[:], ps[:], cs[:, t0:t0 + NT])
                x2 = rtmp.tile([128, 512], f16, tag="x2")
                nc.gpsimd.tensor_mul(x2[:], s_t[:], css[:, t0:t0 + NT])
                return x1, x2

            def emit_rope_b(c, tt, which, x1, x2):
                """Deferred fp8 add - emitted one instance behind stage a so
                the gpsimd x2 mul never stalls the DVE queue."""
                dst = ensure_qk(c)[which]
                t0 = tt * NT
                nc.vector.tensor_add(dst[:, t0:t0 + NT], x1[:], x2[:])

            def rope_items(c):
                """Pending-queue closures for all 8 rope instances of chunk c,
                with each add deferred two slots behind its producer."""
                items = []
                for tt in range(TT):
                    st = {}
                    for which in (0, 1):
                        def a_fn(c=c, tt=tt, which=which, st=st):
                            st[which] = emit_rope_a(c, tt, which)
                        items.append(a_fn)
                    for which in (0, 1):
                        def b_fn(c=c, tt=tt, which=which, st=st):
                            emit_rope_b(c, tt, which, *st[which])
                        items.append(b_fn)
                return items

            pending = []

            def emit_scores(c, tt, sc):
                rq, rk = qk_tiles[c]
                t0 = tt * NT
                s0 = sc * 128
                dlt = max(0, s0 - t0)
                w = NT - dlt
                sp = ps_s.tile([128, 1024], f32, tag="s")
                for h in range(2):
                    sl = rk[h * 64:(h + 1) * 64, s0:s0 + 128]
                    lhsT = bass.AP(sl.tensor, sl.offset,
                                   [list(sl.ap[0]), [T - s0, 2], [1, 128]])
                    sr = rq[h * 64:(h + 1) * 64, t0 + dlt:t0 + NT]
                    rhs = bass.AP(sr.tensor, sr.offset,
                                  [list(sr.ap[0]), [0, 2], [1, w]])
                    nc.tensor.matmul(sp[:, h * NT + dlt:(h + 1) * NT], lhsT, rhs,
                                     start=True, stop=True, perf_mode=DR,
                                     tile_position=(h * 64, 0))
                return sp

            def emit_exp(tt, sc, sp):
                t0 = tt * NT
                s0 = sc * 128
                dlt = max(0, s0 - t0)
                diag = s0 + 127 > t0
                e_t = (e16_pool if diag else e8_pool).tile(
                    [128, 1024], f16 if diag else f8)
                s3 = sp[:].rearrange("p (a b) -> p a b", a=2)[:, :, dlt:]
                e3 = e_t[:].rearrange("p (a b) -> p a b", a=2)[:, :, dlt:]
                nc.scalar.activation(e3, s3, mybir.ActivationFunctionType.Exp,
                                     bias=0.0, scale=0.125)
                if diag:
                    # keep iff j' >= p; for j' >= 128 that's always true,
                    # so only the first 128 columns need the select
                    e3m = e_t[:].rearrange("p (a b) -> p a b", a=2)[:, :, dlt:dlt + 128]
                    nc.gpsimd.affine_select(
                        out=e3m, in_=e3m,
                        compare_op=mybir.AluOpType.is_ge,
                        fill=0.0, base=0,
                        pattern=[[0, 2], [1, 128]], channel_multiplier=-1)
                return e_t

            def emit_attv(c, tt, sc, e_t, yp):
                t0 = tt * NT
                sc_max = (t0 + NT) // 128
                s0 = sc * 128
                dlt = max(0, s0 - t0)
                w = NT - dlt
                diag = s0 + 127 > t0
                for h in range(2):
                    vc = VS * (2 * c + h)
                    if diag:
                        # fp16-rate hi+lo pair: (v_hi + v_lo)^T e16 exactly
                        for s in range(2):
                            nc.tensor.matmul(
                                yp[:, h * NT + dlt:(h + 1) * NT],
                                v84[:, sc, s, vc: vc + 65],
                                e_t[:, h * NT + dlt:(h + 1) * NT],
                                start=(sc == 0 and s == 0),
                                stop=(sc == sc_max - 1 and s == 1),
                                skip_group_check=True)
                    else:
                        nc.tensor.matmul(
                            yp[:, h * NT + dlt:(h + 1) * NT],
                            v84[:, sc, :, vc: vc + 65],
                            slot_b(e_t[:, h * NT + dlt: (h + 1) * NT], w),
                            start=(sc == 0), stop=(sc == sc_max - 1),
                            perf_mode=DR, skip_group_check=True)

            def emit_norm(c, tt, yp):
                t0 = tt * NT
                rd = small.tile([1, 1024], f16, tag="rd")
                nc.vector.reciprocal(rd[:], yp[64:65, :])
                rbc = small.tile([64, 1024], f16, tag="rbc")
                nc.gpsimd.partition_broadcast(rbc[:], rd[:])
                for h in range(2):
                    nc.vector.tensor_mul(
                        y_all[h * 64:(h + 1) * 64, c * T + t0: c * T + t0 + NT],
                        yp[0:64, h * NT:(h + 1) * NT],
                        rbc[:, h * NT:(h + 1) * NT])

            def emit_attn_chunk(c, on_norm=None):
                """Software-pipelined attention stream for one chunk: scores
                run one s-chunk ahead of att@V so the PE's in-order queue
                never starves the Activation engine's exp pipeline."""
                seq = [(tt, sc) for tt in range(TT)
                       for sc in range((tt + 1) * NT // 128)]
                sps = {}
                ets = {}
                yps = {}
                sps[0] = emit_scores(c, *seq[0])

                def attv_and_norm(j):
                    tt_j, sc_j = seq[j]
                    if sc_j == 0:
                        yp_t = ps_y.tile([65, 1024], f32, tag="y")
                        yps[tt_j] = yp_t
                    emit_attv(c, tt_j, sc_j, ets.pop(j), yps[tt_j])
                    if sc_j == (tt_j + 1) * NT // 128 - 1:
                        emit_norm(c, tt_j, yps.pop(tt_j))
                        if on_norm is not None:
                            on_norm(tt_j)

                for i, (tt, sc) in enumerate(seq):
                    if i + 1 < len(seq):
                        sp_t = emit_scores(c, *seq[i + 1])
                        sps[i + 1] = sp_t
                    ets[i] = emit_exp(tt, sc, sps.pop(i))
                    if i >= 1:
                        attv_and_norm(i - 1)  # att@V one s-chunk behind exp
                    if pending:
                        pending.pop(0)()
                for i in (len(seq) - 1,):
                    attv_and_norm(i)

            def emit_oproj_ct(tt, ct):
                t0 = tt * NT
                po = ps_big.tile([128, 512], f32, tag="big")
                for c in range(NCHUNK):
                    nc.tensor.matmul(po[:], wo[:, c * 1024 + ct * 128: c * 1024 + ct * 128 + 128],
                                     y_all[:, c * T + t0: c * T + t0 + NT],
                                     start=(c == 0), stop=(c == NCHUNK - 1))
                ob = osb.tile([128, 512], f16)
                nc.vector.tensor_copy(ob[:], po[:])
                nc.sync.dma_start(ot_d[ct * 128:(ct + 1) * 128, t0:t0 + NT], ob[:])

            # ---- head phase: chunk-0 rope tt0 first, then first v-slices ----
            with nc.named_scope("head"):
                st = {}
                for which in (0, 1):
                    st[which] = emit_rope_a(0, 0, which)
                for which in (0, 1):
                    emit_rope_b(0, 0, which, *st[which])
                for m in range(4):
                    emit_vproj_m(m)
                st = {}
                for which in (0, 1):
                    st[which] = emit_rope_a(0, 1, which)
                for which in (0, 1):
                    emit_rope_b(0, 1, which, *st[which])

            # ---- pending helper work, drained one item per s-chunk of the
            # Act-bound attention inner loop (deadlines commented) ----
            rope0 = rope_items(0)[8:]   # chunk-0 tt2/tt3 (8 items)
            rope1 = rope_items(1)
            # c0 queue: vproj m4..m15 + rope(c0,tt2/tt3) + rope(c1).
            # m_k is popped >= (k-4) slots in; attV(c0) first reads m_k at
            # global slot >= k (4*tt slots precede the tt that reads it), and
            # rope(c0,tt2) B-items sit at index <= 11 < 12 slots before tt2.
            pending.extend([lambda m=m: emit_vproj_m(m) for m in (4, 5)])
            pending.extend(rope0[0:2])
            pending.extend([lambda m=m: emit_vproj_m(m) for m in (6, 7)])
            pending.extend(rope0[2:4])
            pending.extend([lambda m=m: emit_vproj_m(m) for m in (8, 9)])
            pending.extend(rope0[4:6])
            pending.extend([lambda m=m: emit_vproj_m(m) for m in (10, 11)])
            pending.extend(rope0[6:8])
            pending.extend([lambda m=m: emit_vproj_m(m) for m in (12, 13, 14, 15)])
            pending.extend(rope1)

            def queue_oproj(tt):
                if tt < 3:  # tt3 runs in the tail
                    pending.extend(
                        [lambda ct=ct, tt=tt: emit_oproj_ct(tt, ct)
                         for ct in range(8)])

            for c in range(NCHUNK):
                with nc.named_scope(f"attn{c}"):
                    if c in (1, 2):
                        pending.extend(rope_items(c + 1))
                    emit_attn_chunk(
                        c, on_norm=queue_oproj if c == NCHUNK - 1 else None)

            with nc.named_scope("tail"):
                while pending:
                    pending.pop(0)()
                for ct in range(8):
                    emit_oproj_ct(3, ct)

    nc.compile()
    return nc


def _fp8_split(a, np8, scale=1.0):
    """scale*a -> (hi, lo) fp8 with hi + lo ~= scale*a to ~0.13%.

    The scale lifts 0.02-magnitude weights above fp8e4's 2^-9 subnormal
    floor so the lo residual can actually represent the hi rounding error.
    """
    a = np.asarray(a, dtype=np.float32) * scale
    hi = a.astype(np8)
    lo = (a - hi.astype(np.float32)).astype(np8)
    return hi, lo


def _prep_inputs(x, qkv_w, qkv_b):
    """Build the per-core input maps (all host-side numpy)."""
    from concourse import mybir
    np8 = mybir.dt.np(mybir.dt.float8e4)

    x = np.asarray(x, dtype=np.float32)
    qkv_w = np.asarray(qkv_w, dtype=np.float32)
    qkv_b = np.asarray(qkv_b, dtype=np.float32)

    # x8 per batch: hi [10*128, T] (ktiles 8/9 = ones row), lo [8*128, T]
    x8hs, x8ls = [], []
    for b in range(B):
        xh = np.zeros((10 * 128, T), dtype=np8)
        xl = np.zeros((KT * 128, T), dtype=np8)
        hi, lo = _fp8_split(x[b].T, np8)
        xh[:C] = hi
        xl[:C] = lo
        xh[C] = np8(1.0)        # aug ktile 8: ones row
        xh[9 * 128] = np8(1.0)  # aug ktile 9: duplicate ones row
        x8hs.append(xh)
        x8ls.append(xl)

    r = np.arange(64)
    d_r = 2 * ((r // 32) * 16 + (r % 16)) + ((r % 32) >= 16)  # row -> head dim
    p = np.arange(128)
    f_p = ((p // 32) % 2) * 16 + (p % 16)

    ins_g = []
    for g in range(2):
        # wqk8h: [p, kc*1024 + c*256 + which*128 + m]; kc8/9 = bias hi/lo on row 0
        wqkh = np.zeros((128, 10 * 1024), dtype=np8)
        for c in range(NCHUNK):
            for which in range(2):  # 0=q, 1=k
                rows = np.concatenate([
                    which * C + (8 * g + 2 * c + hh) * 64 + d_r for hh in range(2)
                ])  # 128 feature rows
                blk = qkv_w[rows, :]          # (128 feat, 1024 k)
                cm = c * 256 + which * 128
                for kc in range(KT):
                    hi, _lo = _fp8_split(blk[:, kc * 128:(kc + 1) * 128].T, np8, SQ)
                    wqkh[:, kc * 1024 + cm: kc * 1024 + cm + 128] = hi
                bh, bl = _fp8_split(qkv_b[rows], np8, SQ)
                wqkh[0, 8 * 1024 + cm: 8 * 1024 + cm + 128] = bh
                wqkh[0, 9 * 1024 + cm: 9 * 1024 + cm + 128] = bl
        # wv8: [p, kc*VW + VS*h + j]; kc8 = aug hi (bias+ones), kc9 = aug lo (bias)
        wva = np.zeros((KT * 128, VW), dtype=np.float32)
        aug = np.zeros((128, VW), dtype=np.float32)
        for h in range(HPG):
            rows = 2 * C + (8 * g + h) * 64 + np.arange(64)
            wva[:, VS * h: VS * h + 64] = qkv_w[rows, :].T
            aug[0, VS * h: VS * h + 64] = qkv_b[rows]
            aug[0, VS * h + 64] = 1.0
        wvh = np.zeros((128, 10 * VW), dtype=np8)
        wvl = np.zeros((128, KT * VW), dtype=np8)
        for kc in range(KT):
            hi, lo = _fp8_split(wva[kc * 128:(kc + 1) * 128], np8, SV)
            wvh[:, kc * VW:(kc + 1) * VW] = hi
            wvl[:, kc * VW:(kc + 1) * VW] = lo
        augh, augl = _fp8_split(aug, np8, SV)  # ones col becomes SV (exact in fp8)
        augl[0, VS * np.arange(HPG) + 64] = np8(0.0)  # ones col only in hi
        wvh[:, 8 * VW: 9 * VW] = augh
        wvh[:, 9 * VW: 10 * VW] = augl
        ins_g.append((wqkh, wvh, wvl))

    # rope tables (divided by SQ to undo the qk weight prescale)
    inv_freq = (1.0 / (ROPE_BASE ** (np.arange(0, D, 2) / D))).astype(np.float64)
    t = np.arange(T, dtype=np.float64)
    ang = t[None, :] * inv_freq[f_p][:, None]          # (128, T)
    cs = (np.cos(ang) / SQ).astype(np.float16)
    sgn = np.where((p % 32) < 16, -1.0, 1.0)[:, None]
    css = (sgn * np.sin(ang) / SQ).astype(np.float16)

    return x8hs, x8ls, ins_g, cs, css


def _prep_wo(out_w, g):
    out_w = np.asarray(out_w, dtype=np.float32)
    wo = np.empty((128, NCHUNK * 1024), dtype=np.float16)
    for c in range(NCHUNK):
        rows = np.concatenate([(8 * g + 2 * c + hh) * 64 + np.arange(64) for hh in range(2)])
        wo[:, c * 1024:(c + 1) * 1024] = out_w[:, rows].astype(np.float16).T
    return wo


def _build_in_maps(x, qkv_w, qkv_b, out_w):
    x8hs, x8ls, ins_g, cs, css = _prep_inputs(x, qkv_w, qkv_b)
    wos = [_prep_wo(out_w, g) for g in range(2)]
    in_maps = []
    for core in range(N_CORES):
        b, g = core // 2, core % 2
        wqkh, wvh, wvl = ins_g[g]
        in_maps.append({
            "x8h": x8hs[b], "x8l": x8ls[b],
            "wqk8h": wqkh,
            "wv8h": wvh, "wv8l": wvl,
            "wo": wos[g], "cs": cs, "css": css,
        })
    return in_maps


def kernel(x, qkv_w, qkv_b, out_w, out_b):
    from concourse.bass_utils import run_bass_kernel_spmd

    if "nc" not in _CACHE:
        _CACHE["nc"] = _build_nc()
    nc = _CACHE["nc"]

    in_maps = _build_in_maps(x, qkv_w, qkv_b, out_w)
    out_b = np.asarray(out_b, dtype=np.float32)

    try:
        res = run_bass_kernel_spmd(nc, in_maps, core_ids=list(range(N_CORES)))
    except ModuleNotFoundError:
        # BASS_TRACE set but the NTFF profile hook isn't importable here
        import os
        os.environ["BASS_NEVER_TRACE"] = "1"
        res = run_bass_kernel_spmd(nc, in_maps, core_ids=list(range(N_CORES)))

    out = np.empty((B, T, C), dtype=np.float32)
    for b in range(B):
        pt = (res.results[2 * b]["ot"].astype(np.float32)
              + res.results[2 * b + 1]["ot"].astype(np.float32))  # (C, T)
        out[b] = pt.T + out_b[None, :]
    return out


# revision 40
# speedup vs baseline: 1.1555x; 1.0158x over previous
"""Causal self-attention (B=4, T=2048, C=1024, H=16, D=64) on 8 TRN2 NeuronCores.

Sharding: core = (batch b, head-group g) with b = core // 2, g = core % 2.
Each core computes heads [8g, 8g+8) of batch b and produces the partial
out-projection (C, T) fp16 for its head group; the host sums the two
head-group partials per batch and adds the output bias.

Speed structure (vs the all-fp16 baseline):
- Projections and scores run as fp8e4 DoubleRow matmuls (0.5 cycles/row,
  two k-tiles per pass). Weight accuracy is restored with a hi/lo split
  (hi = fp8(S*w), lo = fp8(S*w - hi)); weights are prescaled (SQ=32 for
  qkv, SV=8 for V) to clear fp8e4's 2^-9 subnormal floor. SQ is undone
  in the rope cos/sin tables; SV cancels in softmax normalization since
  the denominator ones-column carries the same scale.
- Scores use a "zero slot": lhsT k-group 1 points at a zeroed column
  block of the rq/rk tile, so a K=64 fp16-shaped matmul still gets the
  DoubleRow rate. q/k are cast to fp8 by the rope add.
- att@V: exp() writes fp8 straight from the Activation engine; V is
  stored as interleaved (v_hi, v_lo) fp8 pairs and one DoubleRow matmul
  computes v_hi^T e + v_lo^T e per s-chunk via a stride-0 broadcast of e.
  Diagonal s-chunks (which carry the concentrated softmax weight) use an
  fp16 e and an fp16 V copy instead - that one change cuts the output
  error ~2x while costing only the extra diagonal columns.
- Phases are emission-interleaved so the PE/DVE work of the V projection,
  next chunk's qk+rope, and the output projection hide under the
  Activation-engine-bound attention inner loop.
"""

import numpy as np

B, T, C = 4, 2048, 1024
H, D = 16, 64
N_CORES = 8
HPG = H // 2            # heads per core (group)
NCHUNK = 4              # head-pair chunks per core
KT = 8                  # k-tiles of 128 over C
TT = 4                  # t-tiles of 512 over T
NT = 512                # t tile (matmul N)
VS = 66                 # v column stride per head (64 dims + ones + pad)
VW = HPG * VS           # 528 v columns per k-chunk block
ROPE_BASE = 10000.0
SQ = 32.0               # qk weight prescale (undone via cs/css tables)
SV = 8.0                # v weight prescale (cancels in softmax normalization)

_CACHE = {}


def _build_nc():
    import concourse.bass as bass
    import concourse.tile as tile
    from concourse import bacc, mybir
    from contextlib import ExitStack

    f16 = mybir.dt.float16
    f32 = mybir.dt.float32
    f8 = mybir.dt.float8e4
    DR = mybir.MatmulPerfMode.DoubleRow

    nc = bacc.Bacc(
        "TRN2",
        target_bir_lowering=False,
        debug=False,
        enable_asserts=True,
        num_devices=N_CORES,
    )

    x8h_d = nc.dram_tensor("x8h", (10 * 128, T), f8, kind="ExternalInput").ap()
    x8l_d = nc.dram_tensor("x8l", (KT * 128, T), f8, kind="ExternalInput").ap()
    wqk8h_d = nc.dram_tensor("wqk8h", (128, 10 * 1024), f8, kind="ExternalInput").ap()
    wv8h_d = nc.dram_tensor("wv8h", (128, 10 * VW), f8, kind="ExternalInput").ap()
    wv8l_d = nc.dram_tensor("wv8l", (128, KT * VW), f8, kind="ExternalInput").ap()
    wo_d = nc.dram_tensor("wo", (128, NCHUNK * 1024), f16, kind="ExternalInput").ap()
    cs_d = nc.dram_tensor("cs", (128, T), f16, kind="ExternalInput").ap()
    css_d = nc.dram_tensor("css", (128, T), f16, kind="ExternalInput").ap()
    ot_d = nc.dram_tensor("ot", (1024, T), f16, kind="ExternalOutput").ap()

    SHUF = list(range(16, 32)) + list(range(0, 16))

    def slot_b(ap, w):
        # [K, 2, w] view with a stride-0 k-group dim (broadcast the same block)
        return bass.AP(ap.tensor, ap.offset, [list(ap.ap[0]), [0, 2], [1, w]])

    with tile.TileContext(nc) as tc:
        with ExitStack() as ctx, nc.allow_low_precision("fp8 attention pipeline"):
            consts = ctx.enter_context(tc.tile_pool(name="consts", bufs=1))
            qk_pool = ctx.enter_context(tc.tile_pool(name="qk", bufs=2))
            rtmp = ctx.enter_context(tc.tile_pool(name="rtmp", bufs=4))
            e8_pool = ctx.enter_context(tc.tile_pool(name="e8", bufs=8))
            e16_pool = ctx.enter_context(tc.tile_pool(name="e16", bufs=6))
            small = ctx.enter_context(tc.tile_pool(name="small", bufs=3))
            osb = ctx.enter_context(tc.tile_pool(name="osb", bufs=6))
            ps_big = ctx.enter_context(tc.tile_pool(name="psbig", bufs=2, space="PSUM"))
            ps_s = ctx.enter_context(tc.tile_pool(name="pss", bufs=2, space="PSUM"))
            ps_y = ctx.enter_context(tc.tile_pool(name="psy", bufs=1, space="PSUM"))

            # ---- resident tiles + input DMA ----
            # issue order follows first use: qk weights + x (t-half 0) feed the
            # head rope chain, then wv for vproj, then the rest
            x8h = consts.tile([128, 10 * T], f8)
            x8l = consts.tile([128, KT * T], f8)
            wqk8h = consts.tile([128, 10 * 1024], f8)
            wv8h = consts.tile([128, 10 * VW], f8)
            wv8l = consts.tile([128, KT * VW], f8)
            cs = consts.tile([128, T], f16)
            css = consts.tile([128, T], f16)
            wo = consts.tile([128, NCHUNK * 1024], f16)

            def dma_split(dst, src, width, parts):
                step = width // parts
                for i in range(parts):
                    nc.sync.dma_start(dst[:, i * step:(i + 1) * step],
                                      src[:, i * step:(i + 1) * step])

            def x_half(half):
                t0, t1 = half * (T // 2), (half + 1) * (T // 2)
                for kc in range(10):
                    nc.sync.dma_start(x8h[:, kc * T + t0: kc * T + t1],
                                      x8h_d[kc * 128:(kc + 1) * 128, t0:t1])
                for kc in range(KT):
                    nc.sync.dma_start(x8l[:, kc * T + t0: kc * T + t1],
                                      x8l_d[kc * 128:(kc + 1) * 128, t0:t1])

            dma_split(wqk8h, wqk8h_d, 10 * 1024, 5)
            x_half(0)
            dma_split(cs, cs_d, T, 2)
            dma_split(css, css_d, T, 2)
            dma_split(wv8h, wv8h_d, 10 * VW, 2)
            dma_split(wv8l, wv8l_d, KT * VW, 2)
            x_half(1)
            dma_split(wo, wo_d, NCHUNK * 1024, 4)

            v8 = consts.tile([128, 16 * 2 * VW], f8)    # (m, hi|lo, col)
            y_all = consts.tile([128, NCHUNK * T], f16)

            x8h3 = x8h[:].rearrange("p (kc t) -> p kc t", kc=10)
            x8l3 = x8l[:].rearrange("p (kc t) -> p kc t", kc=KT)
            wv8h3 = wv8h[:].rearrange("p (kc c) -> p kc c", kc=10)
            wv8l3 = wv8l[:].rearrange("p (kc c) -> p kc c", kc=KT)
            wqk8h3 = wqk8h[:].rearrange("p (kc c) -> p kc c", kc=10)
            v84 = v8[:].rearrange("p (m s c) -> p m s c", m=16, s=2)

            # ---- emission helpers (phases are interleaved for overlap) ----
            HVW = VW // 2   # 264: v columns for 4 heads incl. their ones cols

            def emit_vproj_m(m):
                # two half-width PSUM tiles split at the head-4 boundary so
                # vproj never touches the scores pool (ps_s)
                psa0 = ps_big.tile([128, HVW], f32, tag="big")
                psa1 = ps_big.tile([128, HVW], f32, tag="big")
                psa = (psa0, psa1)
                mm = []
                for j in range(KT // 2):
                    kk = slice(2 * j, 2 * j + 2)
                    mm.append((x8h3[:, kk, m * 128:(m + 1) * 128], wv8h3[:, kk, :]))
                    mm.append((x8l3[:, kk, m * 128:(m + 1) * 128], wv8h3[:, kk, :]))
                    mm.append((x8h3[:, kk, m * 128:(m + 1) * 128], wv8l3[:, kk, :]))
                mm.append((x8h3[:, 8:10, m * 128:(m + 1) * 128], wv8h3[:, 8:10, :]))
                n = len(mm)
                for i, (lhsT, rhs) in enumerate(mm):
                    for half in range(2):
                        nc.tensor.matmul(psa[half][:], lhsT,
                                         rhs[:, :, half * HVW:(half + 1) * HVW],
                                         start=(i == 0), stop=(i == n - 1),
                                         perf_mode=DR)
                base = m * 2 * VW
                for half in range(2):
                    o = half * HVW
                    nc.vector.tensor_copy(v8[:, base + o: base + o + HVW], psa[half][:])
                    nc.vector.tensor_tensor(
                        out=v8[:, base + VW + o: base + VW + o + HVW],
                        in0=psa[half][:], in1=v8[:, base + o: base + o + HVW],
                        op=mybir.AluOpType.subtract)

            qk_tiles = {}

            def ensure_qk(c):
                if c not in qk_tiles:
                    rq = qk_pool.tile([128, T + 128], f8, tag="rq")
                    rk = qk_pool.tile([128, T + 128], f8, tag="rk")
                    nc.gpsimd.memset(rq[:, T:T + 128], 0.0)
                    nc.gpsimd.memset(rk[:, T:T + 128], 0.0)
                    qk_tiles[c] = (rq, rk)
                return qk_tiles[c]

            def emit_rope_a(c, tt, which):
                """Projection + shuffle + the two rope muls (x2 on gpsimd)."""
                dst = ensure_qk(c)[which]
                t0 = tt * NT
                cm = c * 256 + which * 128
                ps = ps_big.tile([128, 512], f32, tag="big")
                mm = []
                for j in range(KT // 2):
                    kk = slice(2 * j, 2 * j + 2)
                    mm.append((wqk8h3[:, kk, cm:cm + 128], x8h3[:, kk, t0:t0 + NT]))
                    mm.append((wqk8h3[:, kk, cm:cm + 128], x8l3[:, kk, t0:t0 + NT]))
                mm.append((wqk8h3[:, 8:10, cm:cm + 128], x8h3[:, 8:10, t0:t0 + NT]))
                n = len(mm)
                for i, (lhsT, rhs) in enumerate(mm):
                    nc.tensor.matmul(ps[:], lhsT, rhs,
                                     start=(i == 0), stop=(i == n - 1), perf_mode=DR)
                s_t = rtmp.tile([128, 512], f32, tag="s")
                nc.vector.stream_shuffle(s_t[:], ps[:], SHUF)
                x1 = rtmp.tile([128, 512], f16, tag="x1")
                nc.vector.tensor_mul(x1[:], ps[:], cs[:, t0:t0 + NT])
                x2 = rtmp.tile([128, 512], f16, tag="x2")
                nc.gpsimd.tensor_mul(x2[:], s_t[:], css[:, t0:t0 + NT])
                return x1, x2

            def emit_rope_b(c, tt, which, x1, x2):
                """Deferred fp8 add - emitted one instance behind stage a so
                the gpsimd x2 mul never stalls the DVE queue."""
                dst = ensure_qk(c)[which]
                t0 = tt * NT
                nc.vector.tensor_add(dst[:, t0:t0 + NT], x1[:], x2[:])

            def rope_items(c):
                """Pending-queue closures for all 8 rope instances of chunk c,
                with each add deferred two slots behind its producer."""
                items = []
                for tt in range(TT):
                    st = {}
                    for which in (0, 1):
                        def a_fn(c=c, tt=tt, which=which, st=st):
                            st[which] = emit_rope_a(c, tt, which)
                        items.append(a_fn)
                    for which in (0, 1):
                        def b_fn(c=c, tt=tt, which=which, st=st):
                            emit_rope_b(c, tt, which, *st[which])
                        items.append(b_fn)
                return items

            pending = []

            def emit_scores(c, tt, sc):
                rq, rk = qk_tiles[c]
                t0 = tt * NT
                s0 = sc * 128
                dlt = max(0, s0 - t0)
                w = NT - dlt
                sp = ps_s.tile([128, 1024], f32, tag="s")
                for h in range(2):
                    sl = rk[h * 64:(h + 1) * 64, s0:s0 + 128]
                    lhsT = bass.AP(sl.tensor, sl.offset,
                                   [list(sl.ap[0]), [T - s0, 2], [1, 128]])
                    sr = rq[h * 64:(h + 1) * 64, t0 + dlt:t0 + NT]
                    rhs = bass.AP(sr.tensor, sr.offset,
                                  [list(sr.ap[0]), [0, 2], [1, w]])
                    nc.tensor.matmul(sp[:, h * NT + dlt:(h + 1) * NT], lhsT, rhs,
                                     start=True, stop=True, perf_mode=DR,
                                     tile_position=(h * 64, 0))
                return sp

            def emit_exp(tt, sc, sp):
                t0 = tt * NT
                s0 = sc * 128
                dlt = max(0, s0 - t0)
                diag = s0 + 127 > t0
                e_t = (e16_pool if diag else e8_pool).tile(
                    [128, 1024], f16 if diag else f8)
                s3 = sp[:].rearrange("p (a b) -> p a b", a=2)[:, :, dlt:]
                e3 = e_t[:].rearrange("p (a b) -> p a b", a=2)[:, :, dlt:]
                nc.scalar.activation(e3, s3, mybir.ActivationFunctionType.Exp,
                                     bias=0.0, scale=0.125)
                if diag:
                    # keep iff j' >= p; for j' >= 128 that's always true,
                    # so only the first 128 columns need the select
                    e3m = e_t[:].rearrange("p (a b) -> p a b", a=2)[:, :, dlt:dlt + 128]
                    nc.gpsimd.affine_select(
                        out=e3m, in_=e3m,
                        compare_op=mybir.AluOpType.is_ge,
                        fill=0.0, base=0,
                        pattern=[[0, 2], [1, 128]], channel_multiplier=-1)
                return e_t

            def emit_attv(c, tt, sc, e_t, yp):
                t0 = tt * NT
                sc_max = (t0 + NT) // 128
                s0 = sc * 128
                dlt = max(0, s0 - t0)
                w = NT - dlt
                diag = s0 + 127 > t0
                for h in range(2):
                    vc = VS * (2 * c + h)
                    if diag:
                        # fp16-rate hi+lo pair: (v_hi + v_lo)^T e16 exactly
                        for s in range(2):
                            nc.tensor.matmul(
                                yp[:, h * NT + dlt:(h + 1) * NT],
                                v84[:, sc, s, vc: vc + 65],
                                e_t[:, h * NT + dlt:(h + 1) * NT],
                                start=(sc == 0 and s == 0),
                                stop=(sc == sc_max - 1 and s == 1),
                                skip_group_check=True)
                    else:
                        nc.tensor.matmul(
                            yp[:, h * NT + dlt:(h + 1) * NT],
                            v84[:, sc, :, vc: vc + 65],
                            slot_b(e_t[:, h * NT + dlt: (h + 1) * NT], w),
                            start=(sc == 0), stop=(sc == sc_max - 1),
                            perf_mode=DR, skip_group_check=True)

            def emit_norm(c, tt, yp):
                t0 = tt * NT
                rd = small.tile([1, 1024], f16, tag="rd")
                nc.vector.reciprocal(rd[:], yp[64:65, :])
                rbc = small.tile([64, 1024], f16, tag="rbc")
                nc.gpsimd.partition_broadcast(rbc[:], rd[:])
                for h in range(2):
                    nc.vector.tensor_mul(
                        y_all[h * 64:(h + 1) * 64, c * T + t0: c * T + t0 + NT],
                        yp[0:64, h * NT:(h + 1) * NT],
                        rbc[:, h * NT:(h + 1) * NT])

            def emit_attn_chunk(c, on_norm=None):
                """Software-pipelined attention stream for one chunk: scores
                run one s-chunk ahead of att@V so the PE's in-order queue
                never starves the Activation engine's exp pipeline."""
                seq = [(tt, sc) for tt in range(TT)
                       for sc in range((tt + 1) * NT // 128)]
                sps = {}
                ets = {}
                yps = {}
                sps[0] = emit_scores(c, *seq[0])

                def attv_and_norm(j):
                    tt_j, sc_j = seq[j]
                    if sc_j == 0:
                        yp_t = ps_y.tile([65, 1024], f32, tag="y")
                        yps[tt_j] = yp_t
                    emit_attv(c, tt_j, sc_j, ets.pop(j), yps[tt_j])
                    if sc_j == (tt_j + 1) * NT // 128 - 1:
                        emit_norm(c, tt_j, yps.pop(tt_j))
                        if on_norm is not None:
                            on_norm(tt_j)

                for i, (tt, sc) in enumerate(seq):
                    if i + 1 < len(seq):
                        sp_t = emit_scores(c, *seq[i + 1])
                        sps[i + 1] = sp_t
                    ets[i] = emit_exp(tt, sc, sps.pop(i))
                    if i >= 2:
                        attv_and_norm(i - 2)  # att@V two s-chunks behind exp
                    if pending:
                        pending.pop(0)()
                for i in (len(seq) - 2, len(seq) - 1):
                    attv_and_norm(i)

            def emit_oproj_ct(tt, ct):
                t0 = tt * NT
                po = ps_big.tile([128, 512], f32, tag="big")
                for c in range(NCHUNK):
                    nc.tensor.matmul(po[:], wo[:, c * 1024 + ct * 128: c * 1024 + ct * 128 + 128],
                                     y_all[:, c * T + t0: c * T + t0 + NT],
                                     start=(c == 0), stop=(c == NCHUNK - 1))
                ob = osb.tile([128, 512], f16)
                nc.vector.tensor_copy(ob[:], po[:])
                nc.sync.dma_start(ot_d[ct * 128:(ct + 1) * 128, t0:t0 + NT], ob[:])

            # ---- head phase: chunk-0 rope tt0 first, then first v-slices ----
            with nc.named_scope("head"):
                st = {}
                for which in (0, 1):
                    st[which] = emit_rope_a(0, 0, which)
                for which in (0, 1):
                    emit_rope_b(0, 0, which, *st[which])
                for m in range(4):
                    emit_vproj_m(m)
                st = {}
                for which in (0, 1):
                    st[which] = emit_rope_a(0, 1, which)
                for which in (0, 1):
                    emit_rope_b(0, 1, which, *st[which])

            # ---- pending helper work, drained one item per s-chunk of the
            # Act-bound attention inner loop (deadlines commented) ----
            rope0 = rope_items(0)[8:]   # chunk-0 tt2/tt3 (8 items)
            rope1 = rope_items(1)
            # c0 queue: vproj m4..m15 + rope(c0,tt2/tt3) + rope(c1).
            # m_k is popped >= (k-4) slots in; attV(c0) first reads m_k at
            # global slot >= k (4*tt slots precede the tt that reads it), and
            # rope(c0,tt2) B-items sit at index <= 11 < 12 slots before tt2.
            pending.extend([lambda m=m: emit_vproj_m(m) for m in (4, 5)])
            pending.extend(rope0[0:2])
            pending.extend([lambda m=m: emit_vproj_m(m) for m in (6, 7)])
            pending.extend(rope0[2:4])
            pending.extend([lambda m=m: emit_vproj_m(m) for m in (8, 9)])
            pending.extend(rope0[4:6])
            pending.extend([lambda m=m: emit_vproj_m(m) for m in (10, 11)])
            pending.extend(rope0[6:8])
            pending.extend([lambda m=m: emit_vproj_m(m) for m in (12, 13, 14, 15)])
            pending.extend(rope1)

            def queue_oproj(tt):
                if tt < 3:  # tt3 runs in the tail
                    pending.extend(
                        [lambda ct=ct, tt=tt: emit_oproj_ct(tt, ct)
                         for ct in range(8)])

            for c in range(NCHUNK):
                with nc.named_scope(f"attn{c}"):
                    if c in (1, 2):
                        pending.extend(rope_items(c + 1))
                    emit_attn_chunk(
                        c, on_norm=queue_oproj if c == NCHUNK - 1 else None)

            with nc.named_scope("tail"):
                while pending:
                    pending.pop(0)()
                for ct in range(8):
                    emit_oproj_ct(3, ct)

    nc.compile()
    return nc


def _fp8_split(a, np8, scale=1.0):
    """scale*a -> (hi, lo) fp8 with hi + lo ~= scale*a to ~0.13%.

    The scale lifts 0.02-magnitude weights above fp8e4's 2^-9 subnormal
    floor so the lo residual can actually represent the hi rounding error.
    """
    a = np.asarray(a, dtype=np.float32) * scale
    hi = a.astype(np8)
    lo = (a - hi.astype(np.float32)).astype(np8)
    return hi, lo


def _prep_inputs(x, qkv_w, qkv_b):
    """Build the per-core input maps (all host-side numpy)."""
    from concourse import mybir
    np8 = mybir.dt.np(mybir.dt.float8e4)

    x = np.asarray(x, dtype=np.float32)
    qkv_w = np.asarray(qkv_w, dtype=np.float32)
    qkv_b = np.asarray(qkv_b, dtype=np.float32)

    # x8 per batch: hi [10*128, T] (ktiles 8/9 = ones row), lo [8*128, T]
    x8hs, x8ls = [], []
    for b in range(B):
        xh = np.zeros((10 * 128, T), dtype=np8)
        xl = np.zeros((KT * 128, T), dtype=np8)
        hi, lo = _fp8_split(x[b].T, np8)
        xh[:C] = hi
        xl[:C] = lo
        xh[C] = np8(1.0)        # aug ktile 8: ones row
        xh[9 * 128] = np8(1.0)  # aug ktile 9: duplicate ones row
        x8hs.append(xh)
        x8ls.append(xl)

    r = np.arange(64)
    d_r = 2 * ((r // 32) * 16 + (r % 16)) + ((r % 32) >= 16)  # row -> head dim
    p = np.arange(128)
    f_p = ((p // 32) % 2) * 16 + (p % 16)

    ins_g = []
    for g in range(2):
        # wqk8h: [p, kc*1024 + c*256 + which*128 + m]; kc8/9 = bias hi/lo on row 0
        wqkh = np.zeros((128, 10 * 1024), dtype=np8)
        for c in range(NCHUNK):
            for which in range(2):  # 0=q, 1=k
                rows = np.concatenate([
                    which * C + (8 * g + 2 * c + hh) * 64 + d_r for hh in range(2)
                ])  # 128 feature rows
                blk = qkv_w[rows, :]          # (128 feat, 1024 k)
                cm = c * 256 + which * 128
                for kc in range(KT):
                    hi, _lo = _fp8_split(blk[:, kc * 128:(kc + 1) * 128].T, np8, SQ)
                    wqkh[:, kc * 1024 + cm: kc * 1024 + cm + 128] = hi
                bh, bl = _fp8_split(qkv_b[rows], np8, SQ)
                wqkh[0, 8 * 1024 + cm: 8 * 1024 + cm + 128] = bh
                wqkh[0, 9 * 1024 + cm: 9 * 1024 + cm + 128] = bl
        # wv8: [p, kc*VW + VS*h + j]; kc8 = aug hi (bias+ones), kc9 = aug lo (bias)
        wva = np.zeros((KT * 128, VW), dtype=np.float32)
        aug = np.zeros((128, VW), dtype=np.float32)
        for h in range(HPG):
            rows = 2 * C + (8 * g + h) * 64 + np.arange(64)
            wva[:, VS * h: VS * h + 64] = qkv_w[rows, :].T
            aug[0, VS * h: VS * h + 64] = qkv_b[rows]
            aug[0, VS * h + 64] = 1.0
        wvh = np.zeros((128, 10 * VW), dtype=np8)
        wvl = np.zeros((128, KT * VW), dtype=np8)
        for kc in range(KT):
            hi, lo = _fp8_split(wva[kc * 128:(kc + 1) * 128], np8, SV)
            wvh[:, kc * VW:(kc + 1) * VW] = hi
            wvl[:, kc * VW:(kc + 1) * VW] = lo
        augh, augl = _fp8_split(aug, np8, SV)  # ones col becomes SV (exact in fp8)
        augl[0, VS * np.arange(HPG) + 64] = np8(0.0)  # ones col only in hi
        wvh[:, 8 * VW: 9 * VW] = augh
        wvh[:, 9 * VW: 10 * VW] = augl
        ins_g.append((wqkh, wvh, wvl))

    # rope tables (divided by SQ to undo the qk weight prescale)
    inv_freq = (1.0 / (ROPE_BASE ** (np.arange(0, D, 2) / D))).astype(np.float64)
    t = np.arange(T, dtype=np.float64)
    ang = t[None, :] * inv_freq[f_p][:, None]          # (128, T)
    cs = (np.cos(ang) / SQ).astype(np.float16)
    sgn = np.where((p % 32) < 16, -1.0, 1.0)[:, None]
    css = (sgn * np.sin(ang) / SQ).astype(np.float16)

    return x8hs, x8ls, ins_g, cs, css


def _prep_wo(out_w, g):
    out_w = np.asarray(out_w, dtype=np.float32)
    wo = np.empty((128, NCHUNK * 1024), dtype=np.float16)
    for c in range(NCHUNK):
        rows = np.concatenate([(8 * g + 2 * c + hh) * 64 + np.arange(64) for hh in range(2)])
        wo[:, c * 1024:(c + 1) * 1024] = out_w[:, rows].astype(np.float16).T
    return wo


def _build_in_maps(x, qkv_w, qkv_b, out_w):
    x8hs, x8ls, ins_g, cs, css = _prep_inputs(x, qkv_w, qkv_b)
    wos = [_prep_wo(out_w, g) for g in range(2)]
    in_maps = []
    for core in range(N_CORES):
        b, g = core // 2, core % 2
        wqkh, wvh, wvl = ins_g[g]
        in_maps.append({
            "x8h": x8hs[b], "x8l": x8ls[b],
            "wqk8h": wqkh,
            "wv8h": wvh, "wv8l": wvl,
            "wo": wos[g], "cs": cs, "css": css,
        })
    return in_maps


def kernel(x, qkv_w, qkv_b, out_w, out_b):
    from concourse.bass_utils import run_bass_kernel_spmd

    if "nc" not in _CACHE:
        _CACHE["nc"] = _build_nc()
    nc = _CACHE["nc"]

    in_maps = _build_in_maps(x, qkv_w, qkv_b, out_w)
    out_b = np.asarray(out_b, dtype=np.float32)

    try:
        res = run_bass_kernel_spmd(nc, in_maps, core_ids=list(range(N_CORES)))
    except ModuleNotFoundError:
        # BASS_TRACE set but the NTFF profile hook isn't importable here
        import os
        os.environ["BASS_NEVER_TRACE"] = "1"
        res = run_bass_kernel_spmd(nc, in_maps, core_ids=list(range(N_CORES)))

    out = np.empty((B, T, C), dtype=np.float32)
    for b in range(B):
        pt = (res.results[2 * b]["ot"].astype(np.float32)
              + res.results[2 * b + 1]["ot"].astype(np.float32))  # (C, T)
        out[b] = pt.T + out_b[None, :]
    return out


# revision 41
# speedup vs baseline: 1.1615x; 1.0052x over previous
"""Causal self-attention (B=4, T=2048, C=1024, H=16, D=64) on 8 TRN2 NeuronCores.

Sharding: core = (batch b, head-group g) with b = core // 2, g = core % 2.
Each core computes heads [8g, 8g+8) of batch b and produces the partial
out-projection (C, T) fp16 for its head group; the host sums the two
head-group partials per batch and adds the output bias.

Speed structure (vs the all-fp16 baseline):
- Projections and scores run as fp8e4 DoubleRow matmuls (0.5 cycles/row,
  two k-tiles per pass). Weight accuracy is restored with a hi/lo split
  (hi = fp8(S*w), lo = fp8(S*w - hi)); weights are prescaled (SQ=32 for
  qkv, SV=8 for V) to clear fp8e4's 2^-9 subnormal floor. SQ is undone
  in the rope cos/sin tables; SV cancels in softmax normalization since
  the denominator ones-column carries the same scale.
- Scores use a "zero slot": lhsT k-group 1 points at a zeroed column
  block of the rq/rk tile, so a K=64 fp16-shaped matmul still gets the
  DoubleRow rate. q/k are cast to fp8 by the rope add.
- att@V: exp() writes fp8 straight from the Activation engine; V is
  stored as interleaved (v_hi, v_lo) fp8 pairs and one DoubleRow matmul
  computes v_hi^T e + v_lo^T e per s-chunk via a stride-0 broadcast of e.
  Diagonal s-chunks (which carry the concentrated softmax weight) use an
  fp16 e and an fp16 V copy instead - that one change cuts the output
  error ~2x while costing only the extra diagonal columns.
- Phases are emission-interleaved so the PE/DVE work of the V projection,
  next chunk's qk+rope, and the output projection hide under the
  Activation-engine-bound attention inner loop.
"""

import numpy as np

B, T, C = 4, 2048, 1024
H, D = 16, 64
N_CORES = 8
HPG = H // 2            # heads per core (group)
NCHUNK = 4              # head-pair chunks per core
KT = 8                  # k-tiles of 128 over C
TT = 4                  # t-tiles of 512 over T
NT = 512                # t tile (matmul N)
VS = 66                 # v column stride per head (64 dims + ones + pad)
VW = HPG * VS           # 528 v columns per k-chunk block
ROPE_BASE = 10000.0
SQ = 32.0               # qk weight prescale (undone via cs/css tables)
SV = 8.0                # v weight prescale (cancels in softmax normalization)

_CACHE = {}


def _build_nc():
    import concourse.bass as bass
    import concourse.tile as tile
    from concourse import bacc, mybir
    from contextlib import ExitStack

    f16 = mybir.dt.float16
    f32 = mybir.dt.float32
    f8 = mybir.dt.float8e4
    DR = mybir.MatmulPerfMode.DoubleRow

    nc = bacc.Bacc(
        "TRN2",
        target_bir_lowering=False,
        debug=False,
        enable_asserts=True,
        num_devices=N_CORES,
    )

    x8h_d = nc.dram_tensor("x8h", (10 * 128, T), f8, kind="ExternalInput").ap()
    x8l_d = nc.dram_tensor("x8l", (KT * 128, T), f8, kind="ExternalInput").ap()
    wqk8h_d = nc.dram_tensor("wqk8h", (128, 10 * 1024), f8, kind="ExternalInput").ap()
    wv8h_d = nc.dram_tensor("wv8h", (128, 10 * VW), f8, kind="ExternalInput").ap()
    wv8l_d = nc.dram_tensor("wv8l", (128, KT * VW), f8, kind="ExternalInput").ap()
    wo_d = nc.dram_tensor("wo", (128, NCHUNK * 1024), f16, kind="ExternalInput").ap()
    cs_d = nc.dram_tensor("cs", (128, T), f16, kind="ExternalInput").ap()
    css_d = nc.dram_tensor("css", (128, T), f16, kind="ExternalInput").ap()
    ot_d = nc.dram_tensor("ot", (1024, T), f16, kind="ExternalOutput").ap()

    SHUF = list(range(16, 32)) + list(range(0, 16))

    def slot_b(ap, w):
        # [K, 2, w] view with a stride-0 k-group dim (broadcast the same block)
        return bass.AP(ap.tensor, ap.offset, [list(ap.ap[0]), [0, 2], [1, w]])

    with tile.TileContext(nc) as tc:
        with ExitStack() as ctx, nc.allow_low_precision("fp8 attention pipeline"):
            consts = ctx.enter_context(tc.tile_pool(name="consts", bufs=1))
            qk_pool = ctx.enter_context(tc.tile_pool(name="qk", bufs=2))
            rtmp = ctx.enter_context(tc.tile_pool(name="rtmp", bufs=6))
            e8_pool = ctx.enter_context(tc.tile_pool(name="e8", bufs=10))
            e16_pool = ctx.enter_context(tc.tile_pool(name="e16", bufs=8))
            small = ctx.enter_context(tc.tile_pool(name="small", bufs=3))
            osb = ctx.enter_context(tc.tile_pool(name="osb", bufs=8))
            ps_big = ctx.enter_context(tc.tile_pool(name="psbig", bufs=2, space="PSUM"))
            ps_s = ctx.enter_context(tc.tile_pool(name="pss", bufs=2, space="PSUM"))
            ps_y = ctx.enter_context(tc.tile_pool(name="psy", bufs=1, space="PSUM"))

            # ---- resident tiles + input DMA ----
            # issue order follows first use: qk weights + x (t-half 0) feed the
            # head rope chain, then wv for vproj, then the rest
            x8h = consts.tile([128, 10 * T], f8)
            x8l = consts.tile([128, KT * T], f8)
            wqk8h = consts.tile([128, 10 * 1024], f8)
            wv8h = consts.tile([128, 10 * VW], f8)
            wv8l = consts.tile([128, KT * VW], f8)
            cs = consts.tile([128, T], f16)
            css = consts.tile([128, T], f16)
            wo = consts.tile([128, NCHUNK * 1024], f16)

            def dma_split(dst, src, width, parts):
                step = width // parts
                for i in range(parts):
                    nc.sync.dma_start(dst[:, i * step:(i + 1) * step],
                                      src[:, i * step:(i + 1) * step])

            def x_half(half):
                t0, t1 = half * (T // 2), (half + 1) * (T // 2)
                for kc in range(10):
                    nc.sync.dma_start(x8h[:, kc * T + t0: kc * T + t1],
                                      x8h_d[kc * 128:(kc + 1) * 128, t0:t1])
                for kc in range(KT):
                    nc.sync.dma_start(x8l[:, kc * T + t0: kc * T + t1],
                                      x8l_d[kc * 128:(kc + 1) * 128, t0:t1])

            dma_split(wqk8h, wqk8h_d, 10 * 1024, 5)
            x_half(0)
            dma_split(cs, cs_d, T, 2)
            dma_split(css, css_d, T, 2)
            dma_split(wv8h, wv8h_d, 10 * VW, 2)
            dma_split(wv8l, wv8l_d, KT * VW, 2)
            x_half(1)
            dma_split(wo, wo_d, NCHUNK * 1024, 4)

            v8 = consts.tile([128, 16 * 2 * VW], f8)    # (m, hi|lo, col)
            y_all = consts.tile([128, NCHUNK * T], f16)

            x8h3 = x8h[:].rearrange("p (kc t) -> p kc t", kc=10)
            x8l3 = x8l[:].rearrange("p (kc t) -> p kc t", kc=KT)
            wv8h3 = wv8h[:].rearrange("p (kc c) -> p kc c", kc=10)
            wv8l3 = wv8l[:].rearrange("p (kc c) -> p kc c", kc=KT)
            wqk8h3 = wqk8h[:].rearrange("p (kc c) -> p kc c", kc=10)
            v84 = v8[:].rearrange("p (m s c) -> p m s c", m=16, s=2)

            # ---- emission helpers (phases are interleaved for overlap) ----
            HVW = VW // 2   # 264: v columns for 4 heads incl. their ones cols

            def emit_vproj_m(m):
                # two half-width PSUM tiles split at the head-4 boundary so
                # vproj never touches the scores pool (ps_s)
                psa0 = ps_big.tile([128, HVW], f32, tag="big")
                psa1 = ps_big.tile([128, HVW], f32, tag="big")
                psa = (psa0, psa1)
                mm = []
                for j in range(KT // 2):
                    kk = slice(2 * j, 2 * j + 2)
                    mm.append((x8h3[:, kk, m * 128:(m + 1) * 128], wv8h3[:, kk, :]))
                    mm.append((x8l3[:, kk, m * 128:(m + 1) * 128], wv8h3[:, kk, :]))
                    mm.append((x8h3[:, kk, m * 128:(m + 1) * 128], wv8l3[:, kk, :]))
                mm.append((x8h3[:, 8:10, m * 128:(m + 1) * 128], wv8h3[:, 8:10, :]))
                n = len(mm)
                for i, (lhsT, rhs) in enumerate(mm):
                    for half in range(2):
                        nc.tensor.matmul(psa[half][:], lhsT,
                                         rhs[:, :, half * HVW:(half + 1) * HVW],
                                         start=(i == 0), stop=(i == n - 1),
                                         perf_mode=DR)
                base = m * 2 * VW
                for half in range(2):
                    o = half * HVW
                    nc.vector.tensor_copy(v8[:, base + o: base + o + HVW], psa[half][:])
                    nc.vector.tensor_tensor(
                        out=v8[:, base + VW + o: base + VW + o + HVW],
                        in0=psa[half][:], in1=v8[:, base + o: base + o + HVW],
                        op=mybir.AluOpType.subtract)

            qk_tiles = {}

            def ensure_qk(c):
                if c not in qk_tiles:
                    rq = qk_pool.tile([128, T + 128], f8, tag="rq")
                    rk = qk_pool.tile([128, T + 128], f8, tag="rk")
                    nc.gpsimd.memset(rq[:, T:T + 128], 0.0)
                    nc.gpsimd.memset(rk[:, T:T + 128], 0.0)
                    qk_tiles[c] = (rq, rk)
                return qk_tiles[c]

            def emit_rope_a(c, tt, which):
                """Projection + shuffle + the two rope muls (x2 on gpsimd)."""
                dst = ensure_qk(c)[which]
                t0 = tt * NT
                cm = c * 256 + which * 128
                ps = ps_big.tile([128, 512], f32, tag="big")
                mm = []
                for j in range(KT // 2):
                    kk = slice(2 * j, 2 * j + 2)
                    mm.append((wqk8h3[:, kk, cm:cm + 128], x8h3[:, kk, t0:t0 + NT]))
                    mm.append((wqk8h3[:, kk, cm:cm + 128], x8l3[:, kk, t0:t0 + NT]))
                mm.append((wqk8h3[:, 8:10, cm:cm + 128], x8h3[:, 8:10, t0:t0 + NT]))
                n = len(mm)
                for i, (lhsT, rhs) in enumerate(mm):
                    nc.tensor.matmul(ps[:], lhsT, rhs,
                                     start=(i == 0), stop=(i == n - 1), perf_mode=DR)
                s_t = rtmp.tile([128, 512], f32, tag="s")
                nc.vector.stream_shuffle(s_t[:], ps[:], SHUF)
                x1 = rtmp.tile([128, 512], f16, tag="x1")
                nc.vector.tensor_mul(x1[:], ps[:], cs[:, t0:t0 + NT])
                x2 = rtmp.tile([128, 512], f16, tag="x2")
                nc.gpsimd.tensor_mul(x2[:], s_t[:], css[:, t0:t0 + NT])
                return x1, x2

            def emit_rope_b(c, tt, which, x1, x2):
                """Deferred fp8 add - emitted one instance behind stage a so
                the gpsimd x2 mul never stalls the DVE queue."""
                dst = ensure_qk(c)[which]
                t0 = tt * NT
                nc.vector.tensor_add(dst[:, t0:t0 + NT], x1[:], x2[:])

            def rope_items(c):
                """Pending-queue closures for all 8 rope instances of chunk c,
                with each add deferred two slots behind its producer."""
                items = []
                for tt in range(TT):
                    st = {}
                    for which in (0, 1):
                        def a_fn(c=c, tt=tt, which=which, st=st):
                            st[which] = emit_rope_a(c, tt, which)
                        items.append(a_fn)
                    for which in (0, 1):
                        def b_fn(c=c, tt=tt, which=which, st=st):
                            emit_rope_b(c, tt, which, *st[which])
                        items.append(b_fn)
                return items

            pending = []

            def emit_scores(c, tt, sc):
                rq, rk = qk_tiles[c]
                t0 = tt * NT
                s0 = sc * 128
                dlt = max(0, s0 - t0)
                w = NT - dlt
                sp = ps_s.tile([128, 1024], f32, tag="s")
                for h in range(2):
                    sl = rk[h * 64:(h + 1) * 64, s0:s0 + 128]
                    lhsT = bass.AP(sl.tensor, sl.offset,
                                   [list(sl.ap[0]), [T - s0, 2], [1, 128]])
                    sr = rq[h * 64:(h + 1) * 64, t0 + dlt:t0 + NT]
                    rhs = bass.AP(sr.tensor, sr.offset,
                                  [list(sr.ap[0]), [0, 2], [1, w]])
                    nc.tensor.matmul(sp[:, h * NT + dlt:(h + 1) * NT], lhsT, rhs,
                                     start=True, stop=True, perf_mode=DR,
                                     tile_position=(h * 64, 0))
                return sp

            def emit_exp(tt, sc, sp):
                t0 = tt * NT
                s0 = sc * 128
                dlt = max(0, s0 - t0)
                diag = s0 + 127 > t0
                e_t = (e16_pool if diag else e8_pool).tile(
                    [128, 1024], f16 if diag else f8)
                s3 = sp[:].rearrange("p (a b) -> p a b", a=2)[:, :, dlt:]
                e3 = e_t[:].rearrange("p (a b) -> p a b", a=2)[:, :, dlt:]
                nc.scalar.activation(e3, s3, mybir.ActivationFunctionType.Exp,
                                     bias=0.0, scale=0.125)
                if diag:
                    # keep iff j' >= p; for j' >= 128 that's always true,
                    # so only the first 128 columns need the select
                    e3m = e_t[:].rearrange("p (a b) -> p a b", a=2)[:, :, dlt:dlt + 128]
                    nc.gpsimd.affine_select(
                        out=e3m, in_=e3m,
                        compare_op=mybir.AluOpType.is_ge,
                        fill=0.0, base=0,
                        pattern=[[0, 2], [1, 128]], channel_multiplier=-1)
                return e_t

            def emit_attv(c, tt, sc, e_t, yp):
                t0 = tt * NT
                sc_max = (t0 + NT) // 128
                s0 = sc * 128
                dlt = max(0, s0 - t0)
                w = NT - dlt
                diag = s0 + 127 > t0
                for h in range(2):
                    vc = VS * (2 * c + h)
                    if diag:
                        # fp16-rate hi+lo pair: (v_hi + v_lo)^T e16 exactly
                        for s in range(2):
                            nc.tensor.matmul(
                                yp[:, h * NT + dlt:(h + 1) * NT],
                                v84[:, sc, s, vc: vc + 65],
                                e_t[:, h * NT + dlt:(h + 1) * NT],
                                start=(sc == 0 and s == 0),
                                stop=(sc == sc_max - 1 and s == 1),
                                skip_group_check=True)
                    else:
                        nc.tensor.matmul(
                            yp[:, h * NT + dlt:(h + 1) * NT],
                            v84[:, sc, :, vc: vc + 65],
                            slot_b(e_t[:, h * NT + dlt: (h + 1) * NT], w),
                            start=(sc == 0), stop=(sc == sc_max - 1),
                            perf_mode=DR, skip_group_check=True)

            def emit_norm(c, tt, yp):
                t0 = tt * NT
                rd = small.tile([1, 1024], f16, tag="rd")
                nc.vector.reciprocal(rd[:], yp[64:65, :])
                rbc = small.tile([64, 1024], f16, tag="rbc")
                nc.gpsimd.partition_broadcast(rbc[:], rd[:])
                for h in range(2):
                    nc.vector.tensor_mul(
                        y_all[h * 64:(h + 1) * 64, c * T + t0: c * T + t0 + NT],
                        yp[0:64, h * NT:(h + 1) * NT],
                        rbc[:, h * NT:(h + 1) * NT])

            def emit_attn_chunk(c, on_norm=None):
                """Software-pipelined attention stream for one chunk: scores
                run one s-chunk ahead of att@V so the PE's in-order queue
                never starves the Activation engine's exp pipeline."""
                seq = [(tt, sc) for tt in range(TT)
                       for sc in range((tt + 1) * NT // 128)]
                sps = {}
                ets = {}
                yps = {}
                sps[0] = emit_scores(c, *seq[0])

                def attv_and_norm(j):
                    tt_j, sc_j = seq[j]
                    if sc_j == 0:
                        yp_t = ps_y.tile([65, 1024], f32, tag="y")
                        yps[tt_j] = yp_t
                    emit_attv(c, tt_j, sc_j, ets.pop(j), yps[tt_j])
                    if sc_j == (tt_j + 1) * NT // 128 - 1:
                        emit_norm(c, tt_j, yps.pop(tt_j))
                        if on_norm is not None:
                            on_norm(tt_j)

                for i, (tt, sc) in enumerate(seq):
                    if i + 1 < len(seq):
                        sp_t = emit_scores(c, *seq[i + 1])
                        sps[i + 1] = sp_t
                    ets[i] = emit_exp(tt, sc, sps.pop(i))
                    if i >= 3:
                        attv_and_norm(i - 3)  # att@V three s-chunks behind exp
                    if pending:
                        pending.pop(0)()
                for i in (len(seq) - 3, len(seq) - 2, len(seq) - 1):
                    attv_and_norm(i)

            def emit_oproj_ct(tt, ct):
                t0 = tt * NT
                po = ps_big.tile([128, 512], f32, tag="big")
                for c in range(NCHUNK):
                    nc.tensor.matmul(po[:], wo[:, c * 1024 + ct * 128: c * 1024 + ct * 128 + 128],
                                     y_all[:, c * T + t0: c * T + t0 + NT],
                                     start=(c == 0), stop=(c == NCHUNK - 1))
                ob = osb.tile([128, 512], f16)
                nc.vector.tensor_copy(ob[:], po[:])
                nc.sync.dma_start(ot_d[ct * 128:(ct + 1) * 128, t0:t0 + NT], ob[:])

            # ---- head phase: chunk-0 rope tt0 first, then first v-slices ----
            with nc.named_scope("head"):
                st = {}
                for which in (0, 1):
                    st[which] = emit_rope_a(0, 0, which)
                for which in (0, 1):
                    emit_rope_b(0, 0, which, *st[which])
                for m in range(4):
                    emit_vproj_m(m)
                st = {}
                for which in (0, 1):
                    st[which] = emit_rope_a(0, 1, which)
                for which in (0, 1):
                    emit_rope_b(0, 1, which, *st[which])

            # ---- pending helper work, drained one item per s-chunk of the
            # Act-bound attention inner loop (deadlines commented) ----
            rope0 = rope_items(0)[8:]   # chunk-0 tt2/tt3 (8 items)
            rope1 = rope_items(1)
            # c0 queue: vproj m4..m15 + rope(c0,tt2/tt3) + rope(c1).
            # m_k is popped >= (k-4) slots in; attV(c0) first reads m_k at
            # global slot >= k (4*tt slots precede the tt that reads it), and
            # rope(c0,tt2) B-items sit at index <= 11 < 12 slots before tt2.
            pending.extend([lambda m=m: emit_vproj_m(m) for m in (4, 5)])
            pending.extend(rope0[0:2])
            pending.extend([lambda m=m: emit_vproj_m(m) for m in (6, 7)])
            pending.extend(rope0[2:4])
            pending.extend([lambda m=m: emit_vproj_m(m) for m in (8, 9)])
            pending.extend(rope0[4:6])
            pending.extend([lambda m=m: emit_vproj_m(m) for m in (10, 11)])
            pending.extend(rope0[6:8])
            pending.extend([lambda m=m: emit_vproj_m(m) for m in (12, 13, 14, 15)])
            pending.extend(rope1)

            def queue_oproj(tt):
                if tt < 3:  # tt3 runs in the tail
                    pending.extend(
                        [lambda ct=ct, tt=tt: emit_oproj_ct(tt, ct)
                         for ct in range(8)])

            for c in range(NCHUNK):
                with nc.named_scope(f"attn{c}"):
                    if c in (1, 2):
                        pending.extend(rope_items(c + 1))
                    emit_attn_chunk(
                        c, on_norm=queue_oproj if c == NCHUNK - 1 else None)

            with nc.named_scope("tail"):
                while pending:
                    pending.pop(0)()
                for ct in range(8):
                    emit_oproj_ct(3, ct)

    nc.compile()
    return nc


def _fp8_split(a, np8, scale=1.0):
    """scale*a -> (hi, lo) fp8 with hi + lo ~= scale*a to ~0.13%.

    The scale lifts 0.02-magnitude weights above fp8e4's 2^-9 subnormal
    floor so the lo residual can actually represent the hi rounding error.
    """
    a = np.asarray(a, dtype=np.float32) * scale
    hi = a.astype(np8)
    lo = (a - hi.astype(np.float32)).astype(np8)
    return hi, lo


def _prep_inputs(x, qkv_w, qkv_b):
    """Build the per-core input maps (all host-side numpy)."""
    from concourse import mybir
    np8 = mybir.dt.np(mybir.dt.float8e4)

    x = np.asarray(x, dtype=np.float32)
    qkv_w = np.asarray(qkv_w, dtype=np.float32)
    qkv_b = np.asarray(qkv_b, dtype=np.float32)

    # x8 per batch: hi [10*128, T] (ktiles 8/9 = ones row), lo [8*128, T]
    x8hs, x8ls = [], []
    for b in range(B):
        xh = np.zeros((10 * 128, T), dtype=np8)
        xl = np.zeros((KT * 128, T), dtype=np8)
        hi, lo = _fp8_split(x[b].T, np8)
        xh[:C] = hi
        xl[:C] = lo
        xh[C] = np8(1.0)        # aug ktile 8: ones row
        xh[9 * 128] = np8(1.0)  # aug ktile 9: duplicate ones row
        x8hs.append(xh)
        x8ls.append(xl)

    r = np.arange(64)
    d_r = 2 * ((r // 32) * 16 + (r % 16)) + ((r % 32) >= 16)  # row -> head dim
    p = np.arange(128)
    f_p = ((p // 32) % 2) * 16 + (p % 16)

    ins_g = []
    for g in range(2):
        # wqk8h: [p, kc*1024 + c*256 + which*128 + m]; kc8/9 = bias hi/lo on row 0
        wqkh = np.zeros((128, 10 * 1024), dtype=np8)
        for c in range(NCHUNK):
            for which in range(2):  # 0=q, 1=k
                rows = np.concatenate([
                    which * C + (8 * g + 2 * c + hh) * 64 + d_r for hh in range(2)
                ])  # 128 feature rows
                blk = qkv_w[rows, :]          # (128 feat, 1024 k)
                cm = c * 256 + which * 128
                for kc in range(KT):
                    hi, _lo = _fp8_split(blk[:, kc * 128:(kc + 1) * 128].T, np8, SQ)
                    wqkh[:, kc * 1024 + cm: kc * 1024 + cm + 128] = hi
                bh, bl = _fp8_split(qkv_b[rows], np8, SQ)
                wqkh[0, 8 * 1024 + cm: 8 * 1024 + cm + 128] = bh
                wqkh[0, 9 * 1024 + cm: 9 * 1024 + cm + 128] = bl
        # wv8: [p, kc*VW + VS*h + j]; kc8 = aug hi (bias+ones), kc9 = aug lo (bias)
        wva = np.zeros((KT * 128, VW), dtype=np.float32)
        aug = np.zeros((128, VW), dtype=np.float32)
        for h in range(HPG):
            rows = 2 * C + (8 * g + h) * 64 + np.arange(64)
            wva[:, VS * h: VS * h + 64] = qkv_w[rows, :].T
            aug[0, VS * h: VS * h + 64] = qkv_b[rows]
            aug[0, VS * h + 64] = 1.0
        wvh = np.zeros((128, 10 * VW), dtype=np8)
        wvl = np.zeros((128, KT * VW), dtype=np8)
        for kc in range(KT):
            hi, lo = _fp8_split(wva[kc * 128:(kc + 1) * 128], np8, SV)
            wvh[:, kc * VW:(kc + 1) * VW] = hi
            wvl[:, kc * VW:(kc + 1) * VW] = lo
        augh, augl = _fp8_split(aug, np8, SV)  # ones col becomes SV (exact in fp8)
        augl[0, VS * np.arange(HPG) + 64] = np8(0.0)  # ones col only in hi
        wvh[:, 8 * VW: 9 * VW] = augh
        wvh[:, 9 * VW: 10 * VW] = augl
        ins_g.append((wqkh, wvh, wvl))

    # rope tables (divided by SQ to undo the qk weight prescale)
    inv_freq = (1.0 / (ROPE_BASE ** (np.arange(0, D, 2) / D))).astype(np.float64)
    t = np.arange(T, dtype=np.float64)
    ang = t[None, :] * inv_freq[f_p][:, None]          # (128, T)
    cs = (np.cos(ang) / SQ).astype(np.float16)
    sgn = np.where((p % 32) < 16, -1.0, 1.0)[:, None]
    css = (sgn * np.sin(ang) / SQ).astype(np.float16)

    return x8hs, x8ls, ins_g, cs, css


def _prep_wo(out_w, g):
    out_w = np.asarray(out_w, dtype=np.float32)
    wo = np.empty((128, NCHUNK * 1024), dtype=np.float16)
    for c in range(NCHUNK):
        rows = np.concatenate([(8 * g + 2 * c + hh) * 64 + np.arange(64) for hh in range(2)])
        wo[:, c * 1024:(c + 1) * 1024] = out_w[:, rows].astype(np.float16).T
    return wo


def _build_in_maps(x, qkv_w, qkv_b, out_w):
    x8hs, x8ls, ins_g, cs, css = _prep_inputs(x, qkv_w, qkv_b)
    wos = [_prep_wo(out_w, g) for g in range(2)]
    in_maps = []
    for core in range(N_CORES):
        b, g = core // 2, core % 2
        wqkh, wvh, wvl = ins_g[g]
        in_maps.append({
            "x8h": x8hs[b], "x8l": x8ls[b],
            "wqk8h": wqkh,
            "wv8h": wvh, "wv8l": wvl,
            "wo": wos[g], "cs": cs, "css": css,
        })
    return in_maps


def kernel(x, qkv_w, qkv_b, out_w, out_b):
    from concourse.bass_utils import run_bass_kernel_spmd

    if "nc" not in _CACHE:
        _CACHE["nc"] = _build_nc()
    nc = _CACHE["nc"]

    in_maps = _build_in_maps(x, qkv_w, qkv_b, out_w)
    out_b = np.asarray(out_b, dtype=np.float32)

    try:
        res = run_bass_kernel_spmd(nc, in_maps, core_ids=list(range(N_CORES)))
    except ModuleNotFoundError:
        # BASS_TRACE set but the NTFF profile hook isn't importable here
        import os
        os.environ["BASS_NEVER_TRACE"] = "1"
        res = run_bass_kernel_spmd(nc, in_maps, core_ids=list(range(N_CORES)))

    out = np.empty((B, T, C), dtype=np.float32)
    for b in range(B):
        pt = (res.results[2 * b]["ot"].astype(np.float32)
              + res.results[2 * b + 1]["ot"].astype(np.float32))  # (C, T)
        out[b] = pt.T + out_b[None, :]
    return out
